# revision 1
# baseline (speedup 1.0000x reference)
"""ChebConv(K=5) + Linear + log_softmax GNN kernel for 8 Trainium2 NeuronCores.

Strategy (graph/data parallel, per sharding hint):
 - Nodes are sharded by destination across 8 cores (6250 nodes each, padded
   to S=6272 rows). Within each core, nodes are permuted so rows are grouped
   by in-degree class; all index structure is precomputed on the host.
 - The normalized propagation  prop(h) = -D^-1/2 A D^-1/2 h  is refactored
   so the device only ever computes raw gather-sums  s[dst] = sum t[src]:
   the table stores t_k = D^-1/2 T_k(L~)x, and the Chebyshev recurrence
   becomes  t_{k+1} = -2 D^-1 s - t_{k-1}  (first hop: t_1 = -D^-1 s).
 - Each hop: every core gathers its edges' source rows from a replicated
   bf16 table in HBM via gpsimd dma_gather (one call per destination tile
   per index range), reduces them per destination with TensorE matmuls
   against precomputed 0/1 "segment" patterns accumulating in PSUM, applies
   the recurrence on VectorE, and contributes its updated slice to the next
   table via an 8-core AllGather (runs on TOPSP/SDMA, overlapped).
 - Because dma_gather indices are int16, the table is split into a "lo"
   region (cores 0-4) addressed from row 0 and a "hi" region (cores 5-7)
   addressed from row 31361; each node's edges are split accordingly.
 - The output projection z = sum_k (sqrt(D) t_k) W_k is accumulated per hop
   (PE transpose + fp32 matmul), followed by relu, the 50->10 Linear and a
   row-wise log_softmax.
"""
import numpy as np
import ml_dtypes

bf16 = ml_dtypes.bfloat16

# ---------------- problem constants (hardcoded per contract) ---------------
N = 50000
E = 1_600_000
D = 128
K = 5
NCORES = 8
NPC = N // NCORES            # 6250
TILES = (NPC + 127) // 128   # 49
S = TILES * 128              # 6272
LO_CORES = 5
LO_SPLIT_NODE = LO_CORES * NPC       # 31250
HI_BASE_ROW = 1 + LO_CORES * S       # 31361
TOT_ROWS = 2 + NCORES * S            # 50178
HI_PAD_IDX = TOT_ROWS - 1 - HI_BASE_ROW  # 18816 -> trailing zero row
ALLOWED_C = np.array([8, 16, 24, 32, 40, 48, 56, 64, 80, 96, 128], dtype=np.int64)
BASE_PID = [1 + S * c for c in range(LO_CORES)] + [
    HI_BASE_ROW + S * (c - LO_CORES) for c in range(LO_CORES, NCORES)
]


def _class_of(d):
    """Smallest allowed class >= d (d: int64 array)."""
    idx = np.searchsorted(ALLOWED_C, d)
    out = ALLOWED_C[np.minimum(idx, len(ALLOWED_C) - 1)]
    assert (out >= d).all(), "degree exceeds max class"
    return np.where(d == 0, 0, out)


def host_prep(x, edge_index):
    row = np.ascontiguousarray(edge_index[0]).astype(np.int64)
    col = np.ascontiguousarray(edge_index[1]).astype(np.int64)
    deg = np.bincount(row, minlength=N)
    assert (deg > 0).all(), "kernel assumes no isolated (deg-0) nodes"
    degf = deg.astype(np.float32)
    dinv = (1.0 / np.sqrt(degf)).astype(np.float32)

    is_lo = col < LO_SPLIT_NODE
    lodeg = np.bincount(row[is_lo], minlength=N)
    hideg = deg - lodeg
    cl = _class_of(lodeg)
    ch = _class_of(hideg)

    # per-core permutation: sort nodes by (cl desc, ch desc)
    perms = np.empty((NCORES, NPC), dtype=np.int64)
    for c in range(NCORES):
        ids = np.arange(c * NPC, (c + 1) * NPC)
        order = np.lexsort((-ch[ids], -cl[ids]))
        perms[c] = ids[order]

    rank = np.empty(N, dtype=np.int64)
    rank[perms.reshape(-1)] = np.tile(np.arange(NPC), NCORES)
    pid = np.asarray(BASE_PID, dtype=np.int64)[np.arange(N) // NPC] + rank

    # common tiling: per tile, max class over all cores (padded rows class 0)
    clp = np.zeros((NCORES, S), dtype=np.int64)
    chp = np.zeros((NCORES, S), dtype=np.int64)
    for c in range(NCORES):
        clp[c, :NPC] = cl[perms[c]]
        chp[c, :NPC] = ch[perms[c]]
    CLO = clp.reshape(NCORES, TILES, 128).max(axis=(0, 2))
    CHI = chp.reshape(NCORES, TILES, 128).max(axis=(0, 2))
    lo_tile_off = np.zeros(TILES + 1, dtype=np.int64)
    hi_tile_off = np.zeros(TILES + 1, dtype=np.int64)
    np.cumsum(CLO * 128, out=lo_tile_off[1:])
    np.cumsum(CHI * 128, out=hi_tile_off[1:])
    n_lo, n_hi = int(lo_tile_off[-1]), int(hi_tile_off[-1])

    # per-node slot bases (in its core's slot array)
    tile_of_rank = np.arange(NPC) // 128
    row_in_tile = np.arange(NPC) % 128
    lo_base_rank = lo_tile_off[tile_of_rank] + row_in_tile * CLO[tile_of_rank]
    hi_base_rank = hi_tile_off[tile_of_rank] + row_in_tile * CHI[tile_of_rank]
    lo_base = np.empty(N, dtype=np.int64)
    hi_base = np.empty(N, dtype=np.int64)
    lo_base[perms.reshape(-1)] = np.tile(lo_base_rank, NCORES)
    hi_base[perms.reshape(-1)] = np.tile(hi_base_rank, NCORES)

    # edges sorted by dst; per-edge rank among same-(dst,pass) edges
    order_e = np.argsort(row, kind="stable")
    row_s, col_s = row[order_e], col[order_e]
    is_lo_s = is_lo[order_e]
    # occurrence index within dst for lo and hi subsets separately
    estart = np.zeros(N + 1, dtype=np.int64)
    np.cumsum(deg, out=estart[1:])

    def occ_index(dst_sub, count_sub):
        st = np.zeros(N + 1, dtype=np.int64)
        np.cumsum(count_sub, out=st[1:])
        return np.arange(dst_sub.shape[0], dtype=np.int64) - st[dst_sub]

    dst_lo, src_lo = row_s[is_lo_s], col_s[is_lo_s]
    dst_hi, src_hi = row_s[~is_lo_s], col_s[~is_lo_s]
    j_lo = occ_index(dst_lo, np.bincount(dst_lo, minlength=N))
    j_hi = occ_index(dst_hi, np.bincount(dst_hi, minlength=N))

    idx_lo = np.zeros((NCORES, n_lo), dtype=np.int16)             # pad -> row 0
    idx_hi = np.full((NCORES, n_hi), HI_PAD_IDX, dtype=np.int16)  # pad -> zero row
    core_lo, core_hi = dst_lo // NPC, dst_hi // NPC
    slot_lo = lo_base[dst_lo] + j_lo
    slot_hi = hi_base[dst_hi] + j_hi
    v_lo = pid[src_lo]
    v_hi = pid[src_hi] - HI_BASE_ROW
    assert v_lo.max() <= 32767 and v_lo.min() >= 1
    assert v_hi.max() <= 32767 and v_hi.min() >= 0
    idx_lo[core_lo, slot_lo] = v_lo.astype(np.int16)
    idx_hi[core_hi, slot_hi] = v_hi.astype(np.int16)

    # wrap to dma_gather layout [128, n/16] (16-partition stripes, 8 replicas)
    def wrap(a):
        t = a.reshape(-1, 16).T          # [16, n/16]
        return np.ascontiguousarray(np.tile(t, (8, 1)))

    idx_lo_w = np.stack([wrap(idx_lo[c]) for c in range(NCORES)])
    idx_hi_w = np.stack([wrap(idx_hi[c]) for c in range(NCORES)])

    # patterns: full-height [128, 128] bf16 one-hot per (class, chunk j)
    pat_pool, pat_list = {}, []
    chunk_meta = {}
    for cval in sorted(set(CLO.tolist()) | set(CHI.tolist())):
        if cval == 0:
            continue
        metas = []
        e = np.arange(128)
        for j in range(int(cval)):
            d = (128 * j + e) // cval
            assert (d < 128).all()
            P = np.zeros((128, 128), dtype=bf16)
            P[e, d] = 1
            key = (int(cval), int(j))
            pat_pool[key] = len(pat_list)
            pat_list.append(P)
            metas.append(pat_pool[key])
        chunk_meta[int(cval)] = metas
    pats = np.stack(pat_list)  # [NPAT, 128, 128]

    # per-row constants in [128, TILES] layout (value for row g at [g%128, g//128])
    def rowconst(vals_percore):  # [NCORES, S] f32 -> [NCORES, 128, TILES]
        return np.ascontiguousarray(
            vals_percore.reshape(NCORES, TILES, 128).transpose(0, 2, 1))

    dinv_p = np.zeros((NCORES, S), dtype=np.float32)
    sdeg_p = np.zeros((NCORES, S), dtype=np.float32)
    for c in range(NCORES):
        dinv_p[c, :NPC] = dinv[perms[c]]
        sdeg_p[c, :NPC] = np.sqrt(degf[perms[c]])
    di2 = dinv_p * dinv_p

    xp = np.zeros((NCORES, S, D), dtype=np.float32)
    for c in range(NCORES):
        xp[c, :NPC] = x[perms[c]]

    return dict(
        perms=perms, CLO=CLO, CHI=CHI,
        lo_tile_off=lo_tile_off, hi_tile_off=hi_tile_off,
        n_lo=n_lo, n_hi=n_hi,
        idx_lo_w=idx_lo_w, idx_hi_w=idx_hi_w,
        pats=pats, chunk_meta=chunk_meta,
        xp=xp,
        dinv_t=rowconst(dinv_p),
        m1di2_t=rowconst(-di2),
        m2di2_t=rowconst(-2.0 * di2),
        sdeg_t=rowconst(sdeg_p),
    )


def build_nc(meta, cheb_w, cheb_b, fc_w, fc_b):
    from concourse import bacc, mybir
    import concourse.tile as tile

    f32, bft, i16 = mybir.dt.float32, mybir.dt.bfloat16, mybir.dt.int16
    CLO, CHI = meta["CLO"], meta["CHI"]
    cm = meta["chunk_meta"]
    n_lo, n_hi = meta["n_lo"], meta["n_hi"]
    NPAT = meta["pats"].shape[0]
    CLO_MAX, CHI_MAX = int(CLO.max()), int(CHI.max())

    nc = bacc.Bacc(target_bir_lowering=False, num_swdge_queues=2)

    # ---- I/O --------------------------------------------------------------
    xp_d = nc.declare_dram_parameter("xp", [S, D], f32, isOutput=False)
    il_d = nc.declare_dram_parameter("idx_lo", [128, n_lo // 16], i16, isOutput=False)
    ih_d = nc.declare_dram_parameter("idx_hi", [128, n_hi // 16], i16, isOutput=False)
    pat_d = nc.declare_dram_parameter("pats", [NPAT * 128, 128], bft, isOutput=False)
    dinv_d = nc.declare_dram_parameter("dinv_t", [128, TILES], f32, isOutput=False)
    m1_d = nc.declare_dram_parameter("m1di2_t", [128, TILES], f32, isOutput=False)
    m2_d = nc.declare_dram_parameter("m2di2_t", [128, TILES], f32, isOutput=False)
    sdeg_d = nc.declare_dram_parameter("sdeg_t", [128, TILES], f32, isOutput=False)
    wch_d = nc.declare_dram_parameter("wcheb", [128, K * 50], f32, isOutput=False)
    cb_d = nc.declare_dram_parameter("cbias", [50, 1], f32, isOutput=False)
    fw_d = nc.declare_dram_parameter("fcw", [50, 10], f32, isOutput=False)
    fb_d = nc.declare_dram_parameter("fcb_rep", [128, 10], f32, isOutput=False)
    id_d = nc.declare_dram_parameter("ident", [128, 128], f32, isOutput=False)
    out_d = nc.declare_dram_parameter("out", [S, 10], f32, isOutput=True)

    # ---- internal DRAM ----------------------------------------------------
    agin = [nc.dram_tensor(f"agin{k}", [S, D], bft) for k in range(K - 1)]
    tables = [
        nc.dram_tensor(f"table{k}", [TOT_ROWS, D], bft, addr_space="Shared")
        for k in range(K - 1)
    ]

    with tile.TileContext(nc) as tc:
        with tc.tile_pool(name="cst", bufs=1) as cst, \
             tc.tile_pool(name="xt", bufs=3) as xtp, \
             tc.tile_pool(name="glo", bufs=2) as glop, \
             tc.tile_pool(name="ghi", bufs=2) as ghip, \
             tc.tile_pool(name="st", bufs=3) as stp, \
             tc.tile_pool(name="fin", bufs=2) as finp, \
             tc.tile_pool(name="ps_s", bufs=2, space="PSUM") as ps_s, \
             tc.tile_pool(name="ps_t", bufs=2, space="PSUM") as ps_t, \
             tc.tile_pool(name="ps_z", bufs=2, space="PSUM") as ps_z:

            # ---- resident constants --------------------------------------
            idx_lo_s = cst.tile([128, n_lo // 16], i16)
            idx_hi_s = cst.tile([128, n_hi // 16], i16)
            nc.sync.dma_start(out=idx_lo_s[:], in_=il_d[:, :])
            nc.sync.dma_start(out=idx_hi_s[:], in_=ih_d[:, :])
            pats_s = cst.tile([128, NPAT, 128], bft)
            nc.sync.dma_start(
                out=pats_s[:],
                in_=pat_d[:, :].rearrange("(n p) d -> p n d", p=128),
            )
            ident = cst.tile([128, 128], f32)
            nc.sync.dma_start(out=ident[:], in_=id_d[:, :])
            dinv_s = cst.tile([128, TILES], f32)
            nc.sync.dma_start(out=dinv_s[:], in_=dinv_d[:, :])
            m1_s = cst.tile([128, TILES], f32)
            nc.sync.dma_start(out=m1_s[:], in_=m1_d[:, :])
            m2_s = cst.tile([128, TILES], f32)
            nc.sync.dma_start(out=m2_s[:], in_=m2_d[:, :])
            sdeg_s = cst.tile([128, TILES], f32)
            nc.sync.dma_start(out=sdeg_s[:], in_=sdeg_d[:, :])
            wch_s = cst.tile([128, K * 50], f32)
            nc.sync.dma_start(out=wch_s[:], in_=wch_d[:, :])
            cb_s = cst.tile([50, 1], f32)
            nc.sync.dma_start(out=cb_s[:], in_=cb_d[:, :])
            fw_s = cst.tile([50, 10], f32)
            nc.sync.dma_start(out=fw_s[:], in_=fw_d[:, :])
            fb_s = cst.tile([128, 10], f32)
            nc.sync.dma_start(out=fb_s[:], in_=fb_d[:, :])

            gA = cst.tile([128, TILES, 128], f32)   # holds t_{k-1} slices
            gB = cst.tile([128, TILES, 128], f32)   # holds t_k slices
            z_s = cst.tile([50, S], f32)            # z^T accumulator

            # zero rows of each table
            zrow = cst.tile([1, D], bft)
            nc.vector.memset(zrow[:], 0.0)
            for t in tables:
                nc.sync.dma_start(out=t[0:1, :], in_=zrow[:])
                nc.sync.dma_start(out=t[TOT_ROWS - 1 : TOT_ROWS, :], in_=zrow[:])

            def z_project(k, src_tile, t):
                """z[:, tile t] (+)= W_k^T @ (sdeg * src_tile)^T"""
                zsc = stp.tile([128, 128], f32, tag="zsc")
                nc.vector.tensor_scalar_mul(
                    out=zsc[:], in0=src_tile, scalar1=sdeg_s[:, t : t + 1])
                tp = ps_t.tile([128, 128], f32, space="PSUM")
                nc.tensor.transpose(out=tp[:], in_=zsc[:], identity=ident[:])
                trs = stp.tile([128, 128], f32, tag="trs")
                nc.vector.tensor_copy(out=trs[:], in_=tp[:])
                zp = ps_z.tile([50, 128], f32, space="PSUM")
                nc.tensor.matmul(out=zp[:], lhsT=wch_s[:, 50 * k : 50 * (k + 1)],
                                 rhs=trs[:], start=True, stop=True)
                zsl = z_s[:, 128 * t : 128 * (t + 1)]
                if k == 0:
                    nc.vector.tensor_copy(out=zsl, in_=zp[:])
                else:
                    nc.vector.tensor_tensor(
                        out=zsl, in0=zsl, in1=zp[:], op=mybir.AluOpType.add)

            # ---- prologue: t_0 = dinv * x, table0, z += W_0 term ---------
            for t in range(TILES):
                xt = xtp.tile([128, D], f32)
                nc.sync.dma_start(out=xt[:], in_=xp_d[128 * t : 128 * (t + 1), :])
                ga = gA[:, t, :]
                nc.vector.tensor_scalar_mul(
                    out=ga, in0=xt[:], scalar1=dinv_s[:, t : t + 1])
                xb = stp.tile([128, D], bft, tag="xb")
                nc.scalar.activation(out=xb[:], in_=ga,
                                     func=mybir.ActivationFunctionType.Copy)
                nc.sync.dma_start(out=agin[0][128 * t : 128 * (t + 1), :], in_=xb[:])
                z_project(0, ga, t)
            nc.gpsimd.collective_compute(
                "AllGather", mybir.AluOpType.bypass,
                replica_groups=[list(range(NCORES))],
                ins=[agin[0][:, :]], outs=[tables[0][1 : TOT_ROWS - 1, :]],
            )

            # ---- hops ----------------------------------------------------
            for k in range(1, K):
                tbl = tables[k - 1]
                tbl_hi = tbl[HI_BASE_ROW:TOT_ROWS, :]
                for t in range(TILES):
                    clo, chi = int(CLO[t]), int(CHI[t])
                    chunks = []  # (pattern_id, G_view)
                    if clo:
                        gl = glop.tile([128, CLO_MAX, 128], bft)
                        o16 = int(meta["lo_tile_off"][t]) // 16
                        nc.gpsimd.dma_gather(
                            out_ap=gl[:, :clo, :],
                            in_ap=tbl[:, :],
                            idxs_ap=idx_lo_s[:, o16 : o16 + clo * 8],
                            num_idxs=clo * 128, num_idxs_reg=clo * 128,
                            elem_size=D, queue_num=0, single_packet=False,
                        )
                        chunks += [(cm[clo][j], gl[:, j, :]) for j in range(clo)]
                    if chi:
                        gh = ghip.tile([128, CHI_MAX, 128], bft)
                        o16 = int(meta["hi_tile_off"][t]) // 16
                        nc.gpsimd.dma_gather(
                            out_ap=gh[:, :chi, :],
                            in_ap=tbl_hi,
                            idxs_ap=idx_hi_s[:, o16 : o16 + chi * 8],
                            num_idxs=chi * 128, num_idxs_reg=chi * 128,
                            elem_size=D, queue_num=1, single_packet=False,
                        )
                        chunks += [(cm[chi][j], gh[:, j, :]) for j in range(chi)]

                    sp = ps_s.tile([128, 128], f32, space="PSUM")
                    nch = len(chunks)
                    for i, (pid_, gv) in enumerate(chunks):
                        nc.tensor.matmul(
                            out=sp[:], lhsT=pats_s[:, pid_, :], rhs=gv,
                            start=(i == 0), stop=(i == nch - 1),
                            skip_group_check=True,
                        )

                    # recurrence
                    dst = gB[:, t, :] if k % 2 == 1 else gA[:, t, :]
                    prv = dst  # t_{k-2} lives in the buffer being overwritten
                    if k == 1:
                        nc.vector.tensor_scalar_mul(
                            out=dst, in0=sp[:], scalar1=m1_s[:, t : t + 1])
                    else:
                        st1 = stp.tile([128, 128], f32, tag="st1")
                        nc.vector.tensor_scalar_mul(
                            out=st1[:], in0=sp[:], scalar1=m2_s[:, t : t + 1])
                        nc.vector.tensor_tensor(
                            out=dst, in0=st1[:], in1=prv,
                            op=mybir.AluOpType.subtract)
                    if k < K - 1:
                        xb = stp.tile([128, D], bft, tag="xb")
                        nc.scalar.activation(out=xb[:], in_=dst,
                                             func=mybir.ActivationFunctionType.Copy)
                        nc.sync.dma_start(
                            out=agin[k][128 * t : 128 * (t + 1), :], in_=xb[:])
                    z_project(k, dst, t)
                if k < K - 1:
                    nc.gpsimd.collective_compute(
                        "AllGather", mybir.AluOpType.bypass,
                        replica_groups=[list(range(NCORES))],
                        ins=[agin[k][:, :]],
                        outs=[tables[k][1 : TOT_ROWS - 1, :]],
                    )

            # ---- final: relu, fc, log_softmax ----------------------------
            for t in range(TILES):
                zsl = z_s[:, 128 * t : 128 * (t + 1)]
                hT = finp.tile([50, 128], f32, tag="hT")
                nc.scalar.activation(out=hT[:], in_=zsl,
                                     func=mybir.ActivationFunctionType.Relu,
                                     bias=cb_s[:, 0:1])
                lgp = ps_z.tile([10, 128], f32, space="PSUM", tag="zp")
                nc.tensor.matmul(out=lgp[:], lhsT=fw_s[:], rhs=hT[:],
                                 start=True, stop=True)
                lgs = finp.tile([10, 128], f32, tag="lgs")
                nc.vector.tensor_copy(out=lgs[:], in_=lgp[:])
                ltp = ps_t.tile([128, 10], f32, space="PSUM", tag="tp")
                nc.tensor.transpose(out=ltp[:], in_=lgs[:],
                                    identity=ident[0:10, 0:10])
                L = finp.tile([128, 10], f32, tag="L")
                nc.vector.tensor_tensor(out=L[:], in0=ltp[:], in1=fb_s[:],
                                        op=mybir.AluOpType.add)
                m = finp.tile([128, 1], f32, tag="m")
                nc.vector.tensor_reduce(out=m[:], in_=L[:],
                                        axis=mybir.AxisListType.X,
                                        op=mybir.AluOpType.max)
                negm = finp.tile([128, 1], f32, tag="negm")
                nc.vector.tensor_scalar_mul(out=negm[:], in0=m[:], scalar1=-1.0)
                Ex = finp.tile([128, 10], f32, tag="Ex")
                ssum = finp.tile([128, 1], f32, tag="ssum")
                nc.scalar.activation(out=Ex[:], in_=L[:],
                                     func=mybir.ActivationFunctionType.Exp,
                                     bias=negm[:, 0:1], accum_out=ssum[:])
                lns = finp.tile([128, 1], f32, tag="lns")
                nc.scalar.activation(out=lns[:], in_=ssum[:],
                                     func=mybir.ActivationFunctionType.Ln)
                O = finp.tile([128, 10], f32, tag="O")
                nc.vector.tensor_scalar(out=O[:], in0=L[:],
                                        scalar1=m[:, 0:1], scalar2=lns[:, 0:1],
                                        op0=mybir.AluOpType.subtract,
                                        op1=mybir.AluOpType.subtract)
                nc.sync.dma_start(out=out_d[128 * t : 128 * (t + 1), :], in_=O[:])
    nc.finalize()
    return nc


_CACHED = {}


def kernel(x, edge_index, cheb_w, cheb_b, fc_w, fc_b):
    x = np.ascontiguousarray(np.asarray(x, dtype=np.float32))
    cheb_w = np.asarray(cheb_w, dtype=np.float32)
    cheb_b = np.asarray(cheb_b, dtype=np.float32)
    fc_w = np.asarray(fc_w, dtype=np.float32)
    fc_b = np.asarray(fc_b, dtype=np.float32)

    meta = host_prep(x, edge_index)
    nc = build_nc(meta, cheb_w, cheb_b, fc_w, fc_b)

    # per-core inputs
    wcheb = np.ascontiguousarray(
        cheb_w.transpose(1, 0, 2).reshape(D, K * 50)).astype(np.float32)
    pats_flat = meta["pats"].reshape(-1, 128)
    in_maps = []
    for c in range(NCORES):
        in_maps.append({
            "xp": meta["xp"][c],
            "idx_lo": meta["idx_lo_w"][c],
            "idx_hi": meta["idx_hi_w"][c],
            "pats": pats_flat,
            "dinv_t": meta["dinv_t"][c],
            "m1di2_t": meta["m1di2_t"][c],
            "m2di2_t": meta["m2di2_t"][c],
            "sdeg_t": meta["sdeg_t"][c],
            "wcheb": wcheb,
            "cbias": cheb_b.reshape(50, 1),
            "fcw": fc_w,
            "fcb_rep": np.tile(fc_b.reshape(1, 10), (128, 1)).astype(np.float32),
            "ident": np.eye(128, dtype=np.float32),
        })

    from concourse.bass_utils import run_bass_kernel_spmd
    res = run_bass_kernel_spmd(nc, in_maps, core_ids=list(range(NCORES)))

    out = np.empty((N, 10), dtype=np.float32)
    for c in range(NCORES):
        out[meta["perms"][c]] = res.results[c]["out"][:NPC]
    return out



# revision 21
# speedup vs baseline: 1.1784x; 1.1784x over previous
"""ChebConv(K=5) + Linear + log_softmax GNN kernel for 8 Trainium2 NeuronCores.

Strategy (graph/data parallel, nodes sharded by destination):
 - Nodes are globally sorted by degree (desc) and dealt round-robin to the 8
   cores (global rank r -> core r%8), so every core's tile t holds nodes of
   near-identical degree.
 - The propagation prop(h) = -D^-1/2 A D^-1/2 h is refactored so the device
   only computes raw gather-sums s[dst] = sum t[src] of the row-scaled table
   t_k = D^-1/2 T_k(L~) x; recurrence: t_{k+1} = -2 D^-1 s - t_{k-1}.
 - Each hop, every core gathers its edges' source rows from a replicated
   bf16 table in HBM via gpsimd dma_gather. Calls rotate over all 4 SWDGE
   queues, which lets the 16 SDMA engines overlap the HBM random-read
   latency of different queues' 256B packets (~128 GB/s vs ~45 single-queue).
 - Slots are interleaved (chunk j, partition s = j-th edge of the tile's
   s-th node), so the per-destination segment sums reduce with an in-place
   VectorE pairwise tree over the chunk axis - node-major, no transposes.
 - int16 gather indices address a 32768-row window; the table stores core
   c's nodes at rows 6400c..6400c+6271 with 128 zero spare rows per core.
   Cores' windows: lo = rows [0, 32768), hi = rows [18432, 51200). Each
   node's edges are split lo/hi, with edges from the overlap region
   assigned to balance the two streams (halves the padding).
 - z = sum_k (sqrt(D) t_k) W_k accumulates per hop (PE transpose + matmul);
   epilogue: relu, 50->10 Linear, row-wise log_softmax.
"""
import numpy as np
import ml_dtypes

bf16 = ml_dtypes.bfloat16

# ---------------- problem constants (hardcoded per contract) ---------------
D = 128
K = 5
NCORES = 8
GMAX_TILES = 2          # tiles per gather call group


def _set_dims(n):
    """Derive layout constants from node count (module globals)."""
    global N, NPC, TILES, S, SPC, TPC, TOT_TOK, WB, HI0, LO_PAD, HI_PAD
    N = n
    NPC = N // NCORES
    TILES = (NPC + 127) // 128
    S = TILES * 128
    SPC = TILES + 1              # row stripes per core (data + 1 zero spare)
    TPC = SPC * 128              # table rows per core
    TOT_TOK = NCORES * TPC
    WB = max(TOT_TOK - 32768, 0)         # hi window base row
    HI0 = min(32768, TOT_TOK)            # lo window end
    assert WB % 128 == 0
    LO_PAD = S                           # row S (core0 spare) = zero row
    HI_PAD = (NCORES - 1) * TPC + S - WB  # core7 spare, hi-window relative
    assert LO_CORES_OK()


def LO_CORES_OK():
    return LO_PAD < HI0 and 0 <= HI_PAD < 32768


_set_dims(50000)


def host_prep(x, edge_index):
    row = np.ascontiguousarray(edge_index[0]).astype(np.int64)
    col = np.ascontiguousarray(edge_index[1]).astype(np.int64)
    deg = np.bincount(row, minlength=N)
    assert (deg > 0).all(), "kernel assumes no isolated (deg-0) nodes"
    degf = deg.astype(np.float32)
    dinv = (1.0 / np.sqrt(degf)).astype(np.float32)

    grank = np.argsort(-deg, kind="stable")
    rank_of = np.empty(N, dtype=np.int64)
    rank_of[grank] = np.arange(N)
    core_of = (rank_of % NCORES).astype(np.int64)
    ric = rank_of // NCORES                  # rank in core, 0..NPC-1
    token = TPC * core_of + ric              # table row per node
    tile_of = ric // 128
    part_of = ric % 128

    # lo/hi split: forced by window coverage; flex edges balance the streams
    st = token[col]
    cat = np.where(st < WB, 0, np.where(st >= HI0, 2, 1))
    forced_lo = np.bincount(row[cat == 0], minlength=N)
    forced_hi = np.bincount(row[cat == 2], minlength=N)
    lo_n = np.clip((deg + 1) // 2, forced_lo, deg - forced_hi)

    order_e = np.argsort(row, kind="stable")
    row_s, col_s, cat_s = row[order_e], col[order_e], cat[order_e]

    def occ(dst_sub, counts):
        stt = np.zeros(N + 1, dtype=np.int64)
        np.cumsum(counts, out=stt[1:])
        return np.arange(dst_sub.shape[0], dtype=np.int64) - stt[dst_sub]

    flex_rank = np.full(len(row_s), -1, dtype=np.int64)
    fm = cat_s == 1
    flex_rank[fm] = occ(row_s[fm], np.bincount(row_s[fm], minlength=N))
    is_lo_e = (cat_s == 0) | (fm & (flex_rank < (lo_n - forced_lo)[row_s]))
    hi_n = deg - lo_n

    # per-tile chunk counts (max over the tile's 128 nodes, all cores)
    C_lo = np.zeros(TILES, dtype=np.int64)
    C_hi = np.zeros(TILES, dtype=np.int64)
    np.maximum.at(C_lo, tile_of, lo_n)
    np.maximum.at(C_hi, tile_of, hi_n)
    C_lo = np.maximum(C_lo, 1)
    C_hi = np.maximum(C_hi, 1)

    # group tiles into gather calls (GMAX_TILES tiles per call)
    groups = [list(range(g, min(g + GMAX_TILES, TILES)))
              for g in range(0, TILES, GMAX_TILES)]

    def call_layout(C):
        tile_chunk0 = np.zeros(TILES, dtype=np.int64)   # chunk offset in call
        call_base = np.zeros(len(groups) + 1, dtype=np.int64)  # slot base
        call_len = np.zeros(len(groups), dtype=np.int64)
        for gi, g in enumerate(groups):
            q = 0
            for t in g:
                tile_chunk0[t] = q
                q += int(C[t])
            call_len[gi] = 128 * q
            call_base[gi + 1] = call_base[gi] + call_len[gi]
        return tile_chunk0, call_base, call_len

    lo_c0, lo_cbase, lo_clen = call_layout(C_lo)
    hi_c0, hi_cbase, hi_clen = call_layout(C_hi)
    n_lo, n_hi = int(lo_cbase[-1]), int(hi_cbase[-1])
    gid_of = np.repeat(np.arange(len(groups)),
                       [len(g) for g in groups])[:TILES]

    # per-node slot base: call_base + 128*(tile_chunk0) + partition
    node_lo_base = lo_cbase[gid_of[tile_of]] + 128 * lo_c0[tile_of] + part_of
    node_hi_base = hi_cbase[gid_of[tile_of]] + 128 * hi_c0[tile_of] + part_of

    dst_lo, src_lo = row_s[is_lo_e], col_s[is_lo_e]
    dst_hi, src_hi = row_s[~is_lo_e], col_s[~is_lo_e]
    j_lo = occ(dst_lo, np.bincount(dst_lo, minlength=N))
    j_hi = occ(dst_hi, np.bincount(dst_hi, minlength=N))

    idx_lo = np.full((NCORES, n_lo), LO_PAD, dtype=np.int16)
    idx_hi = np.full((NCORES, n_hi), HI_PAD, dtype=np.int16)
    v_lo = token[src_lo]
    v_hi = token[src_hi] - WB
    assert v_lo.max() < 32768 and v_lo.min() >= 0
    assert v_hi.max() < 32768 and v_hi.min() >= 0
    idx_lo[core_of[dst_lo], node_lo_base[dst_lo] + 128 * j_lo] = \
        v_lo.astype(np.int16)
    idx_hi[core_of[dst_hi], node_hi_base[dst_hi] + 128 * j_hi] = \
        v_hi.astype(np.int16)

    # wrap each call's indices to the dma_gather [128, n/16] layout
    def wrap_calls(idx, cbase):
        ncol = idx.shape[1] // 16
        out = np.empty((NCORES, 128, ncol), dtype=np.int16)
        for gi in range(len(groups)):
            a, b = int(cbase[gi]), int(cbase[gi + 1])
            seg = idx[:, a:b].reshape(NCORES, -1, 16).transpose(0, 2, 1)
            out[:, :16, a // 16:b // 16] = seg
        out[:, 16:, :] = np.tile(out[:, :16, :], (1, 7, 1))
        return np.ascontiguousarray(out)

    idx_lo_w = wrap_calls(idx_lo, lo_cbase)
    idx_hi_w = wrap_calls(idx_hi, hi_cbase)

    def rowconst(vals):
        out = np.zeros((NCORES, 128, TILES), dtype=np.float32)
        out[core_of, part_of, tile_of] = vals
        return np.ascontiguousarray(out)

    di2 = dinv * dinv
    xp = np.zeros((NCORES, S, D), dtype=np.float32)
    xp[core_of, ric] = x
    perm = np.empty((NCORES, NPC), dtype=np.int64)
    perm[core_of, ric] = np.arange(N)

    return dict(
        perm=perm, C_lo=C_lo, C_hi=C_hi, groups=groups,
        lo_c0=lo_c0, lo_cbase=lo_cbase, lo_clen=lo_clen,
        hi_c0=hi_c0, hi_cbase=hi_cbase, hi_clen=hi_clen,
        n_lo=n_lo, n_hi=n_hi,
        idx_lo_w=idx_lo_w, idx_hi_w=idx_hi_w,
        xp=xp,
        dinv_t=rowconst(dinv),
        m1_t=rowconst(-di2),
        m2_t=rowconst(-2.0 * di2),
        sdeg_t=rowconst(np.sqrt(degf)),
    )


def build_nc(meta):
    from concourse import bacc, mybir
    import concourse.tile as tile

    f32, bft, i16 = mybir.dt.float32, mybir.dt.bfloat16, mybir.dt.int16
    C_lo, C_hi, groups = meta["C_lo"], meta["C_hi"], meta["groups"]
    n_lo, n_hi = meta["n_lo"], meta["n_hi"]
    CG = int(max(meta["lo_clen"].max(), meta["hi_clen"].max())) // 128

    nc = bacc.Bacc(target_bir_lowering=False, num_swdge_queues=4,
                   dynamic_dma_scratch_size=32768)

    # ---- I/O --------------------------------------------------------------
    xp_d = nc.declare_dram_parameter("xp", [S, D], f32, isOutput=False)
    il_d = nc.declare_dram_parameter("idx_lo", [128, n_lo // 16], i16,
                                     isOutput=False)
    ih_d = nc.declare_dram_parameter("idx_hi", [128, n_hi // 16], i16,
                                     isOutput=False)
    dinv_d = nc.declare_dram_parameter("dinv_t", [128, TILES], f32, isOutput=False)
    m1_d = nc.declare_dram_parameter("m1_t", [128, TILES], f32, isOutput=False)
    m2_d = nc.declare_dram_parameter("m2_t", [128, TILES], f32, isOutput=False)
    sdeg_d = nc.declare_dram_parameter("sdeg_t", [128, TILES], f32, isOutput=False)
    wch_d = nc.declare_dram_parameter("wcheb", [128, K * 50], bft, isOutput=False)
    cb_d = nc.declare_dram_parameter("cbias", [50, 1], f32, isOutput=False)
    fw_d = nc.declare_dram_parameter("fcw", [50, 10], bft, isOutput=False)
    fb_d = nc.declare_dram_parameter("fcb_rep", [128, 10], f32, isOutput=False)
    idf_d = nc.declare_dram_parameter("identf", [128, 128], f32, isOutput=False)
    idb_d = nc.declare_dram_parameter("identb", [128, 128], bft, isOutput=False)
    out_d = nc.declare_dram_parameter("out", [S, 10], f32, isOutput=True)

    # ---- internal DRAM ----------------------------------------------------
    agin = [nc.dram_tensor(f"agin{h}", [TPC, D], bft) for h in range(K - 1)]
    tbl = [nc.dram_tensor(f"tbl{h}", [TOT_TOK, D], bft, addr_space="Shared")
           for h in range(K - 1)]

    with tile.TileContext(nc) as tc:
        with tc.tile_pool(name="cst", bufs=1) as cst, \
             tc.tile_pool(name="xt", bufs=3) as xtp, \
             tc.tile_pool(name="gth", bufs=5) as gthp, \
             tc.tile_pool(name="st", bufs=3) as stp, \
             tc.tile_pool(name="sc", bufs=4) as scp, \
             tc.tile_pool(name="fin", bufs=2) as finp, \
             tc.tile_pool(name="ps_zt", bufs=2, space="PSUM") as ps_zt, \
             tc.tile_pool(name="ps_z", bufs=2, space="PSUM") as ps_z, \
             tc.tile_pool(name="ps_e", bufs=1, space="PSUM") as ps_e:

            # ---- resident constants --------------------------------------
            idx_lo_s = cst.tile([128, n_lo // 16], i16)
            nc.sync.dma_start(out=idx_lo_s[:], in_=il_d[:, :])
            idx_hi_s = cst.tile([128, n_hi // 16], i16)
            nc.sync.dma_start(out=idx_hi_s[:], in_=ih_d[:, :])
            dinv_s = cst.tile([128, TILES], f32)
            nc.sync.dma_start(out=dinv_s[:], in_=dinv_d[:, :])
            m1_s = cst.tile([128, TILES], f32)
            nc.sync.dma_start(out=m1_s[:], in_=m1_d[:, :])
            m2_s = cst.tile([128, TILES], f32)
            nc.sync.dma_start(out=m2_s[:], in_=m2_d[:, :])
            sdeg_s = cst.tile([128, TILES], f32)
            nc.sync.dma_start(out=sdeg_s[:], in_=sdeg_d[:, :])
            wch_s = cst.tile([128, K * 50], bft)
            nc.sync.dma_start(out=wch_s[:], in_=wch_d[:, :])
            cb_s = cst.tile([50, 1], f32)
            nc.sync.dma_start(out=cb_s[:], in_=cb_d[:, :])
            fw_s = cst.tile([50, 10], bft)
            nc.sync.dma_start(out=fw_s[:], in_=fw_d[:, :])
            fb_s = cst.tile([128, 10], f32)
            nc.sync.dma_start(out=fb_s[:], in_=fb_d[:, :])
            idf = cst.tile([128, 128], f32)
            nc.sync.dma_start(out=idf[:], in_=idf_d[:, :])
            idb = cst.tile([128, 128], bft)
            nc.sync.dma_start(out=idb[:], in_=idb_d[:, :])

            prevA = cst.tile([128, SPC, 128], bft)     # node-major t storage
            prevB = cst.tile([128, SPC, 128], bft)
            z_s = cst.tile([50, S], bft)               # z^T accumulator

            nc.vector.memset(prevA[:, TILES, :], 0.0)  # spare rows = zeros
            nc.vector.memset(prevB[:, TILES, :], 0.0)

            def z_project(k, src_bf, t):
                """z^T[:, tile t] (+)= W_k^T @ (sdeg * src)^T  (src node-major)"""
                zsc = scp.tile([128, 128], bft, tag="zsc")
                nc.vector.tensor_scalar_mul(
                    out=zsc[:], in0=src_bf, scalar1=sdeg_s[:, t:t + 1])
                ztp = ps_zt.tile([128, 128], bft, space="PSUM", tag="ztp")
                nc.tensor.transpose(out=ztp[:], in_=zsc[:], identity=idb[:])
                zrhs = scp.tile([128, 128], bft, tag="zrhs")
                nc.scalar.activation(out=zrhs[:], in_=ztp[:],
                                     func=mybir.ActivationFunctionType.Copy)
                zp = ps_z.tile([50, 128], f32, space="PSUM", tag="zp")
                nc.tensor.matmul(out=zp[:], lhsT=wch_s[:, 50 * k:50 * (k + 1)],
                                 rhs=zrhs[:], start=True, stop=True)
                zsl = z_s[:, 128 * t:128 * (t + 1)]
                if k == 0:
                    nc.vector.tensor_copy(out=zsl, in_=zp[:])
                else:
                    nc.vector.tensor_tensor(
                        out=zsl, in0=zsl, in1=zp[:], op=mybir.AluOpType.add)

            def tree(buf, q0, C):
                """in-place pairwise sum of chunks [q0, q0+C) -> chunk q0"""
                n = C
                while n > 1:
                    h = (n + 1) // 2
                    nc.vector.tensor_tensor(
                        out=buf[:, q0:q0 + n - h, :],
                        in0=buf[:, q0:q0 + n - h, :],
                        in1=buf[:, q0 + h:q0 + n, :],
                        op=mybir.AluOpType.add)
                    n = h

            # ---- prologue: t_0 = dinv * x --------------------------------
            for t in range(TILES):
                xt = xtp.tile([128, D], f32)
                nc.sync.dma_start(out=xt[:], in_=xp_d[128 * t:128 * (t + 1), :])
                t0b = stp.tile([128, D], bft, tag="nb")
                nc.vector.tensor_scalar_mul(
                    out=t0b[:], in0=xt[:], scalar1=dinv_s[:, t:t + 1])
                nc.scalar.activation(out=prevA[:, t, :], in_=t0b[:],
                                     func=mybir.ActivationFunctionType.Copy)
                z_project(0, t0b[:], t)
            nc.sync.dma_start(
                out=agin[0][:, :].rearrange("(s p) d -> p s d", p=128),
                in_=prevA[:, :, :])
            nc.gpsimd.collective_compute(
                "AllGather", mybir.AluOpType.bypass,
                replica_groups=[list(range(NCORES))],
                ins=[agin[0][:, :]], outs=[tbl[0][:, :]],
            )

            # ---- hops ----------------------------------------------------
            nq = [0]  # emitted-gather counter (queue = count % 4 keeps the
            # tile framework's 8 DMASW lanes aligned with queues)
            for h in range(1, K):
                tb = tbl[h - 1]
                rd = prevA if h % 2 == 0 else prevB
                wr = prevB if h == 1 else rd

                for gi, g in enumerate(groups):
                    glo = gthp.tile([128, CG, 128], bft, tag="g")
                    ghi = gthp.tile([128, CG, 128], bft, tag="g")
                    nlo = int(meta["lo_clen"][gi])
                    nhi = int(meta["hi_clen"][gi])
                    a = int(meta["lo_cbase"][gi]) // 16
                    nc.gpsimd.dma_gather(
                        out_ap=glo[:, 0:nlo // 128, :], in_ap=tb[0:HI0, :],
                        idxs_ap=idx_lo_s[:, a:a + nlo // 16],
                        num_idxs=nlo, num_idxs_reg=nlo,
                        elem_size=D, queue_num=nq[0] % 4, single_packet=False)
                    nq[0] += 1
                    a = int(meta["hi_cbase"][gi]) // 16
                    nc.gpsimd.dma_gather(
                        out_ap=ghi[:, 0:nhi // 128, :], in_ap=tb[WB:TOT_TOK, :],
                        idxs_ap=idx_hi_s[:, a:a + nhi // 16],
                        num_idxs=nhi, num_idxs_reg=nhi,
                        elem_size=D, queue_num=nq[0] % 4, single_packet=False)
                    nq[0] += 1

                    for t in g:
                        cl, ch = int(C_lo[t]), int(C_hi[t])
                        q0l, q0h = int(meta["lo_c0"][t]), int(meta["hi_c0"][t])
                        tree(glo, q0l, cl)
                        tree(ghi, q0h, ch)
                        stt = stp.tile([128, 128], f32, tag="stt")
                        nc.vector.tensor_tensor(
                            out=stt[:], in0=glo[:, q0l, :], in1=ghi[:, q0h, :],
                            op=mybir.AluOpType.add)
                        nb = stp.tile([128, D], bft, tag="nb")
                        if h == 1:
                            nc.vector.tensor_scalar_mul(
                                out=nb[:], in0=stt[:], scalar1=m1_s[:, t:t + 1])
                        else:
                            r1 = stp.tile([128, 128], f32, tag="r1")
                            nc.vector.tensor_scalar_mul(
                                out=r1[:], in0=stt[:], scalar1=m2_s[:, t:t + 1])
                            nc.vector.tensor_tensor(
                                out=nb[:], in0=r1[:], in1=rd[:, t, :],
                                op=mybir.AluOpType.subtract)
                        if h < K - 1:
                            nc.scalar.activation(
                                out=wr[:, t, :], in_=nb[:],
                                func=mybir.ActivationFunctionType.Copy)
                        z_project(h, nb[:], t)

                if h < K - 1:
                    nc.sync.dma_start(
                        out=agin[h][:, :].rearrange("(s p) d -> p s d", p=128),
                        in_=wr[:, :, :])
                    nc.gpsimd.collective_compute(
                        "AllGather", mybir.AluOpType.bypass,
                        replica_groups=[list(range(NCORES))],
                        ins=[agin[h][:, :]], outs=[tbl[h][:, :]],
                    )

            # ---- final: relu, fc, log_softmax ----------------------------
            for t in range(TILES):
                zsl = z_s[:, 128 * t:128 * (t + 1)]
                hT = finp.tile([50, 128], bft, tag="hT")
                nc.scalar.activation(out=hT[:], in_=zsl,
                                     func=mybir.ActivationFunctionType.Relu,
                                     bias=cb_s[:, 0:1])
                lgp = ps_e.tile([10, 128], f32, space="PSUM", tag="lgp")
                nc.tensor.matmul(out=lgp[:], lhsT=fw_s[:], rhs=hT[:],
                                 start=True, stop=True)
                lgs = finp.tile([10, 128], f32, tag="lgs")
                nc.vector.tensor_copy(out=lgs[:], in_=lgp[:])
                ltp = ps_e.tile([128, 10], f32, space="PSUM", tag="ltp")
                nc.tensor.transpose(out=ltp[:], in_=lgs[:],
                                    identity=idf[0:10, 0:10])
                L = finp.tile([128, 10], f32, tag="L")
                nc.vector.tensor_tensor(out=L[:], in0=ltp[:], in1=fb_s[:],
                                        op=mybir.AluOpType.add)
                m = finp.tile([128, 1], f32, tag="m")
                nc.vector.tensor_reduce(out=m[:], in_=L[:],
                                        axis=mybir.AxisListType.X,
                                        op=mybir.AluOpType.max)
                negm = finp.tile([128, 1], f32, tag="negm")
                nc.vector.tensor_scalar_mul(out=negm[:], in0=m[:], scalar1=-1.0)
                Ex = finp.tile([128, 10], f32, tag="Ex")
                ssum = finp.tile([128, 1], f32, tag="ssum")
                nc.scalar.activation(out=Ex[:], in_=L[:],
                                     func=mybir.ActivationFunctionType.Exp,
                                     bias=negm[:, 0:1], accum_out=ssum[:])
                lns = finp.tile([128, 1], f32, tag="lns")
                nc.scalar.activation(out=lns[:], in_=ssum[:],
                                     func=mybir.ActivationFunctionType.Ln)
                O = finp.tile([128, 10], f32, tag="O")
                nc.vector.tensor_scalar(out=O[:], in0=L[:],
                                        scalar1=m[:, 0:1], scalar2=lns[:, 0:1],
                                        op0=mybir.AluOpType.subtract,
                                        op1=mybir.AluOpType.subtract)
                nc.sync.dma_start(out=out_d[128 * t:128 * (t + 1), :], in_=O[:])
    nc.finalize()
    return nc


def make_in_maps(meta, cheb_w, cheb_b, fc_w, fc_b):
    wcheb = np.ascontiguousarray(
        cheb_w.transpose(1, 0, 2).reshape(D, K * 50)).astype(bf16)
    in_maps = []
    for c in range(NCORES):
        in_maps.append({
            "xp": meta["xp"][c],
            "idx_lo": meta["idx_lo_w"][c],
            "idx_hi": meta["idx_hi_w"][c],
            "dinv_t": meta["dinv_t"][c],
            "m1_t": meta["m1_t"][c],
            "m2_t": meta["m2_t"][c],
            "sdeg_t": meta["sdeg_t"][c],
            "wcheb": wcheb,
            "cbias": cheb_b.reshape(50, 1).astype(np.float32),
            "fcw": fc_w.astype(bf16),
            "fcb_rep": np.tile(fc_b.reshape(1, 10), (128, 1)).astype(np.float32),
            "identf": np.eye(128, dtype=np.float32),
            "identb": np.eye(128, dtype=np.float32).astype(bf16),
        })
    return in_maps


def kernel(x, edge_index, cheb_w, cheb_b, fc_w, fc_b):
    x = np.ascontiguousarray(np.asarray(x, dtype=np.float32))
    cheb_w = np.asarray(cheb_w, dtype=np.float32)
    cheb_b = np.asarray(cheb_b, dtype=np.float32)
    fc_w = np.asarray(fc_w, dtype=np.float32)
    fc_b = np.asarray(fc_b, dtype=np.float32)

    meta = host_prep(x, edge_index)
    nc = build_nc(meta)
    in_maps = make_in_maps(meta, cheb_w, cheb_b, fc_w, fc_b)

    from concourse.bass_utils import run_bass_kernel_spmd
    res = run_bass_kernel_spmd(nc, in_maps, core_ids=list(range(NCORES)))

    out = np.empty((N, 10), dtype=np.float32)
    for c in range(NCORES):
        out[meta["perm"][c]] = res.results[c]["out"][:NPC]
    return out


# revision 24
# speedup vs baseline: 1.2347x; 1.0478x over previous
"""ChebConv(K=5) + Linear + log_softmax GNN kernel for 8 Trainium2 NeuronCores.

Strategy (graph/data parallel, nodes sharded by destination):
 - Nodes are globally sorted by degree (desc) and dealt round-robin to the 8
   cores (global rank r -> core r%8), so every core's tile t holds nodes of
   near-identical degree.
 - The propagation prop(h) = -D^-1/2 A D^-1/2 h is refactored so the device
   only computes raw gather-sums s[dst] = sum t[src] of the row-scaled table
   t_k = D^-1/2 T_k(L~) x; recurrence: t_{k+1} = -2 D^-1 s - t_{k-1}.
 - Each hop, every core gathers its edges' source rows from a replicated
   bf16 table in HBM via gpsimd dma_gather. Calls rotate over all 4 SWDGE
   queues, which lets the 16 SDMA engines overlap the HBM random-read
   latency of different queues' 256B packets (~128 GB/s vs ~45 single-queue).
 - Slots are interleaved (chunk j, partition s = j-th edge of the tile's
   s-th node), so the per-destination segment sums reduce with an in-place
   VectorE pairwise tree over the chunk axis - node-major, no transposes.
 - int16 gather indices address a 32768-row window; the table stores core
   c's nodes at rows 6400c..6400c+6271 with 128 zero spare rows per core.
   Cores' windows: lo = rows [0, 32768), hi = rows [18432, 51200). Each
   node's edges are split lo/hi, with edges from the overlap region
   assigned to balance the two streams (halves the padding).
 - z = sum_k (sqrt(D) t_k) W_k accumulates per hop (PE transpose + matmul);
   epilogue: relu, 50->10 Linear, row-wise log_softmax.
"""
import numpy as np
import ml_dtypes

bf16 = ml_dtypes.bfloat16

# ---------------- problem constants (hardcoded per contract) ---------------
D = 128
K = 5
NCORES = 8
GMAX_TILES = 2          # tiles per gather call group


def _set_dims(n):
    """Derive layout constants from node count (module globals)."""
    global N, NPC, TILES, S, SPC, TPC, TOT_TOK, WB, HI0, LO_PAD, HI_PAD
    N = n
    NPC = N // NCORES
    TILES = (NPC + 127) // 128
    S = TILES * 128
    SPC = TILES + 1              # row stripes per core (data + 1 zero spare)
    TPC = SPC * 128              # table rows per core
    TOT_TOK = NCORES * TPC
    WB = max(TOT_TOK - 32768, 0)         # hi window base row
    HI0 = min(32768, TOT_TOK)            # lo window end
    assert WB % 128 == 0
    LO_PAD = S                           # row S (core0 spare) = zero row
    HI_PAD = (NCORES - 1) * TPC + S - WB  # core7 spare, hi-window relative
    assert LO_CORES_OK()


def LO_CORES_OK():
    return LO_PAD < HI0 and 0 <= HI_PAD < 32768


_set_dims(50000)


def host_prep(x, edge_index):
    row = np.ascontiguousarray(edge_index[0]).astype(np.int64)
    col = np.ascontiguousarray(edge_index[1]).astype(np.int64)
    deg = np.bincount(row, minlength=N)
    assert (deg > 0).all(), "kernel assumes no isolated (deg-0) nodes"
    degf = deg.astype(np.float32)
    dinv = (1.0 / np.sqrt(degf)).astype(np.float32)

    grank = np.argsort(-deg, kind="stable")
    rank_of = np.empty(N, dtype=np.int64)
    rank_of[grank] = np.arange(N)
    core_of = (rank_of % NCORES).astype(np.int64)
    ric = rank_of // NCORES                  # rank in core, 0..NPC-1
    token = TPC * core_of + ric              # table row per node
    tile_of = ric // 128
    part_of = ric % 128

    # lo/hi split: forced by window coverage; flex edges balance the streams
    st = token[col]
    cat = np.where(st < WB, 0, np.where(st >= HI0, 2, 1))
    forced_lo = np.bincount(row[cat == 0], minlength=N)
    forced_hi = np.bincount(row[cat == 2], minlength=N)
    lo_n = np.clip((deg + 1) // 2, forced_lo, deg - forced_hi)

    order_e = np.argsort(row, kind="stable")
    row_s, col_s, cat_s = row[order_e], col[order_e], cat[order_e]

    def occ(dst_sub, counts):
        stt = np.zeros(N + 1, dtype=np.int64)
        np.cumsum(counts, out=stt[1:])
        return np.arange(dst_sub.shape[0], dtype=np.int64) - stt[dst_sub]

    flex_rank = np.full(len(row_s), -1, dtype=np.int64)
    fm = cat_s == 1
    flex_rank[fm] = occ(row_s[fm], np.bincount(row_s[fm], minlength=N))
    is_lo_e = (cat_s == 0) | (fm & (flex_rank < (lo_n - forced_lo)[row_s]))
    hi_n = deg - lo_n

    # per-tile chunk counts (max over the tile's 128 nodes, all cores)
    C_lo = np.zeros(TILES, dtype=np.int64)
    C_hi = np.zeros(TILES, dtype=np.int64)
    np.maximum.at(C_lo, tile_of, lo_n)
    np.maximum.at(C_hi, tile_of, hi_n)
    C_lo = np.maximum(C_lo, 1)
    C_hi = np.maximum(C_hi, 1)

    # group tiles into gather calls: pair big-C with small-C tiles so all
    # calls are similar size (tiles are degree-sorted, so pair ends)
    groups = [[t, TILES - 1 - t] for t in range(TILES // 2)]
    if TILES % 2:
        groups.append([TILES // 2])

    def call_layout(C):
        tile_chunk0 = np.zeros(TILES, dtype=np.int64)   # chunk offset in call
        call_base = np.zeros(len(groups) + 1, dtype=np.int64)  # slot base
        call_len = np.zeros(len(groups), dtype=np.int64)
        for gi, g in enumerate(groups):
            q = 0
            for t in g:
                tile_chunk0[t] = q
                q += int(C[t])
            call_len[gi] = 128 * q
            call_base[gi + 1] = call_base[gi] + call_len[gi]
        return tile_chunk0, call_base, call_len

    lo_c0, lo_cbase, lo_clen = call_layout(C_lo)
    hi_c0, hi_cbase, hi_clen = call_layout(C_hi)
    n_lo, n_hi = int(lo_cbase[-1]), int(hi_cbase[-1])
    gid_of = np.empty(TILES, dtype=np.int64)
    for gi, g in enumerate(groups):
        for t in g:
            gid_of[t] = gi

    # per-node slot base: call_base + 128*(tile_chunk0) + partition
    node_lo_base = lo_cbase[gid_of[tile_of]] + 128 * lo_c0[tile_of] + part_of
    node_hi_base = hi_cbase[gid_of[tile_of]] + 128 * hi_c0[tile_of] + part_of

    dst_lo, src_lo = row_s[is_lo_e], col_s[is_lo_e]
    dst_hi, src_hi = row_s[~is_lo_e], col_s[~is_lo_e]
    j_lo = occ(dst_lo, np.bincount(dst_lo, minlength=N))
    j_hi = occ(dst_hi, np.bincount(dst_hi, minlength=N))

    idx_lo = np.full((NCORES, n_lo), LO_PAD, dtype=np.int16)
    idx_hi = np.full((NCORES, n_hi), HI_PAD, dtype=np.int16)
    v_lo = token[src_lo]
    v_hi = token[src_hi] - WB
    assert v_lo.max() < 32768 and v_lo.min() >= 0
    assert v_hi.max() < 32768 and v_hi.min() >= 0
    idx_lo[core_of[dst_lo], node_lo_base[dst_lo] + 128 * j_lo] = \
        v_lo.astype(np.int16)
    idx_hi[core_of[dst_hi], node_hi_base[dst_hi] + 128 * j_hi] = \
        v_hi.astype(np.int16)

    # wrap each call's indices to the dma_gather [128, n/16] layout
    def wrap_calls(idx, cbase):
        ncol = idx.shape[1] // 16
        out = np.empty((NCORES, 128, ncol), dtype=np.int16)
        for gi in range(len(groups)):
            a, b = int(cbase[gi]), int(cbase[gi + 1])
            seg = idx[:, a:b].reshape(NCORES, -1, 16).transpose(0, 2, 1)
            out[:, :16, a // 16:b // 16] = seg
        out[:, 16:, :] = np.tile(out[:, :16, :], (1, 7, 1))
        return np.ascontiguousarray(out)

    idx_lo_w = wrap_calls(idx_lo, lo_cbase)
    idx_hi_w = wrap_calls(idx_hi, hi_cbase)

    def rowconst(vals):
        out = np.zeros((NCORES, 128, TILES), dtype=np.float32)
        out[core_of, part_of, tile_of] = vals
        return np.ascontiguousarray(out)

    di2 = dinv * dinv
    xp = np.zeros((NCORES, S, D), dtype=np.float32)
    xp[core_of, ric] = x
    perm = np.empty((NCORES, NPC), dtype=np.int64)
    perm[core_of, ric] = np.arange(N)

    return dict(
        perm=perm, C_lo=C_lo, C_hi=C_hi, groups=groups,
        lo_c0=lo_c0, lo_cbase=lo_cbase, lo_clen=lo_clen,
        hi_c0=hi_c0, hi_cbase=hi_cbase, hi_clen=hi_clen,
        n_lo=n_lo, n_hi=n_hi,
        idx_lo_w=idx_lo_w, idx_hi_w=idx_hi_w,
        xp=xp,
        dinv_t=rowconst(dinv),
        m1_t=rowconst(-di2),
        m2_t=rowconst(-2.0 * di2),
        sdeg_t=rowconst(np.sqrt(degf)),
    )


def build_nc(meta):
    from concourse import bacc, mybir
    import concourse.tile as tile

    f32, bft, i16 = mybir.dt.float32, mybir.dt.bfloat16, mybir.dt.int16
    C_lo, C_hi, groups = meta["C_lo"], meta["C_hi"], meta["groups"]
    n_lo, n_hi = meta["n_lo"], meta["n_hi"]
    CG = int(max(meta["lo_clen"].max(), meta["hi_clen"].max())) // 128

    nc = bacc.Bacc(target_bir_lowering=False, num_swdge_queues=4,
                   dynamic_dma_scratch_size=32768)

    # ---- I/O --------------------------------------------------------------
    xp_d = nc.declare_dram_parameter("xp", [S, D], f32, isOutput=False)
    il_d = nc.declare_dram_parameter("idx_lo", [128, n_lo // 16], i16,
                                     isOutput=False)
    ih_d = nc.declare_dram_parameter("idx_hi", [128, n_hi // 16], i16,
                                     isOutput=False)
    dinv_d = nc.declare_dram_parameter("dinv_t", [128, TILES], f32, isOutput=False)
    m1_d = nc.declare_dram_parameter("m1_t", [128, TILES], f32, isOutput=False)
    m2_d = nc.declare_dram_parameter("m2_t", [128, TILES], f32, isOutput=False)
    sdeg_d = nc.declare_dram_parameter("sdeg_t", [128, TILES], f32, isOutput=False)
    wch_d = nc.declare_dram_parameter("wcheb", [128, K * 50], bft, isOutput=False)
    cb_d = nc.declare_dram_parameter("cbias", [50, 1], f32, isOutput=False)
    fw_d = nc.declare_dram_parameter("fcw", [50, 10], bft, isOutput=False)
    fb_d = nc.declare_dram_parameter("fcb_rep", [128, 10], f32, isOutput=False)
    idf_d = nc.declare_dram_parameter("identf", [128, 128], f32, isOutput=False)
    idb_d = nc.declare_dram_parameter("identb", [128, 128], bft, isOutput=False)
    out_d = nc.declare_dram_parameter("out", [S, 10], f32, isOutput=True)

    # ---- internal DRAM ----------------------------------------------------
    agin = [nc.dram_tensor(f"agin{h}", [TPC, D], bft) for h in range(K - 1)]
    tbl = [nc.dram_tensor(f"tbl{h}", [TOT_TOK, D], bft, addr_space="Shared")
           for h in range(K - 1)]

    with tile.TileContext(nc) as tc:
        with tc.tile_pool(name="cst", bufs=1) as cst, \
             tc.tile_pool(name="xt", bufs=3) as xtp, \
             tc.tile_pool(name="gth", bufs=8) as gthp, \
             tc.tile_pool(name="st", bufs=3) as stp, \
             tc.tile_pool(name="sc", bufs=4) as scp, \
             tc.tile_pool(name="fin", bufs=2) as finp, \
             tc.tile_pool(name="ps_zt", bufs=2, space="PSUM") as ps_zt, \
             tc.tile_pool(name="ps_z", bufs=2, space="PSUM") as ps_z, \
             tc.tile_pool(name="ps_e", bufs=1, space="PSUM") as ps_e:

            # ---- resident constants --------------------------------------
            idx_lo_s = cst.tile([128, n_lo // 16], i16)
            nc.sync.dma_start(out=idx_lo_s[:], in_=il_d[:, :])
            idx_hi_s = cst.tile([128, n_hi // 16], i16)
            nc.sync.dma_start(out=idx_hi_s[:], in_=ih_d[:, :])
            dinv_s = cst.tile([128, TILES], f32)
            nc.sync.dma_start(out=dinv_s[:], in_=dinv_d[:, :])
            m1_s = cst.tile([128, TILES], f32)
            nc.sync.dma_start(out=m1_s[:], in_=m1_d[:, :])
            m2_s = cst.tile([128, TILES], f32)
            nc.sync.dma_start(out=m2_s[:], in_=m2_d[:, :])
            sdeg_s = cst.tile([128, TILES], f32)
            nc.sync.dma_start(out=sdeg_s[:], in_=sdeg_d[:, :])
            wch_s = cst.tile([128, K * 50], bft)
            nc.sync.dma_start(out=wch_s[:], in_=wch_d[:, :])
            cb_s = cst.tile([50, 1], f32)
            nc.sync.dma_start(out=cb_s[:], in_=cb_d[:, :])
            fw_s = cst.tile([50, 10], bft)
            nc.sync.dma_start(out=fw_s[:], in_=fw_d[:, :])
            fb_s = cst.tile([128, 10], f32)
            nc.sync.dma_start(out=fb_s[:], in_=fb_d[:, :])
            idf = cst.tile([128, 128], f32)
            nc.sync.dma_start(out=idf[:], in_=idf_d[:, :])
            idb = cst.tile([128, 128], bft)
            nc.sync.dma_start(out=idb[:], in_=idb_d[:, :])

            prevA = cst.tile([128, SPC, 128], bft)     # node-major t storage
            prevB = cst.tile([128, SPC, 128], bft)
            z_s = cst.tile([50, S], bft)               # z^T accumulator

            nc.vector.memset(prevA[:, TILES, :], 0.0)  # spare rows = zeros
            nc.vector.memset(prevB[:, TILES, :], 0.0)

            def z_project(k, src_bf, t):
                """z^T[:, tile t] (+)= W_k^T @ (sdeg * src)^T  (src node-major)"""
                zsc = scp.tile([128, 128], bft, tag="zsc")
                nc.vector.tensor_scalar_mul(
                    out=zsc[:], in0=src_bf, scalar1=sdeg_s[:, t:t + 1])
                ztp = ps_zt.tile([128, 128], bft, space="PSUM", tag="ztp")
                nc.tensor.transpose(out=ztp[:], in_=zsc[:], identity=idb[:])
                zrhs = scp.tile([128, 128], bft, tag="zrhs")
                nc.scalar.activation(out=zrhs[:], in_=ztp[:],
                                     func=mybir.ActivationFunctionType.Copy)
                zp = ps_z.tile([50, 128], f32, space="PSUM", tag="zp")
                nc.tensor.matmul(out=zp[:], lhsT=wch_s[:, 50 * k:50 * (k + 1)],
                                 rhs=zrhs[:], start=True, stop=True)
                zsl = z_s[:, 128 * t:128 * (t + 1)]
                if k == 0:
                    nc.vector.tensor_copy(out=zsl, in_=zp[:])
                else:
                    nc.vector.tensor_tensor(
                        out=zsl, in0=zsl, in1=zp[:], op=mybir.AluOpType.add)

            def tree(buf, q0, C):
                """in-place pairwise sum of chunks [q0, q0+C) -> chunk q0"""
                n = C
                while n > 1:
                    h = (n + 1) // 2
                    nc.vector.tensor_tensor(
                        out=buf[:, q0:q0 + n - h, :],
                        in0=buf[:, q0:q0 + n - h, :],
                        in1=buf[:, q0 + h:q0 + n, :],
                        op=mybir.AluOpType.add)
                    n = h

            # ---- prologue: t_0 = dinv * x --------------------------------
            for t in range(TILES):
                xt = xtp.tile([128, D], f32)
                nc.sync.dma_start(out=xt[:], in_=xp_d[128 * t:128 * (t + 1), :])
                t0b = stp.tile([128, D], bft, tag="nb")
                nc.vector.tensor_scalar_mul(
                    out=t0b[:], in0=xt[:], scalar1=dinv_s[:, t:t + 1])
                nc.scalar.activation(out=prevA[:, t, :], in_=t0b[:],
                                     func=mybir.ActivationFunctionType.Copy)
                z_project(0, t0b[:], t)
            nc.sync.dma_start(
                out=agin[0][:, :].rearrange("(s p) d -> p s d", p=128),
                in_=prevA[:, :, :])
            nc.gpsimd.collective_compute(
                "AllGather", mybir.AluOpType.bypass,
                replica_groups=[list(range(NCORES))],
                ins=[agin[0][:, :]], outs=[tbl[0][:, :]],
            )

            # ---- hops ----------------------------------------------------
            nq = [0]  # emitted-gather counter (queue = count % 4 keeps the
            # tile framework's 8 DMASW lanes aligned with queues)
            for h in range(1, K):
                tb = tbl[h - 1]
                rd = prevA if h % 2 == 0 else prevB
                wr = prevB if h == 1 else rd

                for gi, g in enumerate(groups):
                    glo = gthp.tile([128, CG, 128], bft, tag="g")
                    ghi = gthp.tile([128, CG, 128], bft, tag="g")
                    nlo = int(meta["lo_clen"][gi])
                    nhi = int(meta["hi_clen"][gi])
                    a = int(meta["lo_cbase"][gi]) // 16
                    nc.gpsimd.dma_gather(
                        out_ap=glo[:, 0:nlo // 128, :], in_ap=tb[0:HI0, :],
                        idxs_ap=idx_lo_s[:, a:a + nlo // 16],
                        num_idxs=nlo, num_idxs_reg=nlo,
                        elem_size=D, queue_num=nq[0] % 4, single_packet=False)
                    nq[0] += 1
                    a = int(meta["hi_cbase"][gi]) // 16
                    nc.gpsimd.dma_gather(
                        out_ap=ghi[:, 0:nhi // 128, :], in_ap=tb[WB:TOT_TOK, :],
                        idxs_ap=idx_hi_s[:, a:a + nhi // 16],
                        num_idxs=nhi, num_idxs_reg=nhi,
                        elem_size=D, queue_num=nq[0] % 4, single_packet=False)
                    nq[0] += 1

                    for t in g:
                        cl, ch = int(C_lo[t]), int(C_hi[t])
                        q0l, q0h = int(meta["lo_c0"][t]), int(meta["hi_c0"][t])
                        tree(glo, q0l, cl)
                        tree(ghi, q0h, ch)
                        stt = stp.tile([128, 128], f32, tag="stt")
                        nc.vector.tensor_tensor(
                            out=stt[:], in0=glo[:, q0l, :], in1=ghi[:, q0h, :],
                            op=mybir.AluOpType.add)
                        nb = stp.tile([128, D], bft, tag="nb")
                        if h == 1:
                            nc.vector.tensor_scalar_mul(
                                out=nb[:], in0=stt[:], scalar1=m1_s[:, t:t + 1])
                        else:
                            r1 = stp.tile([128, 128], f32, tag="r1")
                            nc.vector.tensor_scalar_mul(
                                out=r1[:], in0=stt[:], scalar1=m2_s[:, t:t + 1])
                            nc.vector.tensor_tensor(
                                out=nb[:], in0=r1[:], in1=rd[:, t, :],
                                op=mybir.AluOpType.subtract)
                        if h < K - 1:
                            nc.scalar.activation(
                                out=wr[:, t, :], in_=nb[:],
                                func=mybir.ActivationFunctionType.Copy)
                        z_project(h, nb[:], t)

                if h < K - 1:
                    nc.sync.dma_start(
                        out=agin[h][:, :].rearrange("(s p) d -> p s d", p=128),
                        in_=wr[:, :, :])
                    nc.gpsimd.collective_compute(
                        "AllGather", mybir.AluOpType.bypass,
                        replica_groups=[list(range(NCORES))],
                        ins=[agin[h][:, :]], outs=[tbl[h][:, :]],
                    )

            # ---- final: relu, fc, log_softmax ----------------------------
            for t in range(TILES):
                zsl = z_s[:, 128 * t:128 * (t + 1)]
                hT = finp.tile([50, 128], bft, tag="hT")
                nc.scalar.activation(out=hT[:], in_=zsl,
                                     func=mybir.ActivationFunctionType.Relu,
                                     bias=cb_s[:, 0:1])
                lgp = ps_e.tile([10, 128], f32, space="PSUM", tag="lgp")
                nc.tensor.matmul(out=lgp[:], lhsT=fw_s[:], rhs=hT[:],
                                 start=True, stop=True)
                lgs = finp.tile([10, 128], f32, tag="lgs")
                nc.vector.tensor_copy(out=lgs[:], in_=lgp[:])
                ltp = ps_e.tile([128, 10], f32, space="PSUM", tag="ltp")
                nc.tensor.transpose(out=ltp[:], in_=lgs[:],
                                    identity=idf[0:10, 0:10])
                L = finp.tile([128, 10], f32, tag="L")
                nc.vector.tensor_tensor(out=L[:], in0=ltp[:], in1=fb_s[:],
                                        op=mybir.AluOpType.add)
                m = finp.tile([128, 1], f32, tag="m")
                nc.vector.tensor_reduce(out=m[:], in_=L[:],
                                        axis=mybir.AxisListType.X,
                                        op=mybir.AluOpType.max)
                negm = finp.tile([128, 1], f32, tag="negm")
                nc.vector.tensor_scalar_mul(out=negm[:], in0=m[:], scalar1=-1.0)
                Ex = finp.tile([128, 10], f32, tag="Ex")
                ssum = finp.tile([128, 1], f32, tag="ssum")
                nc.scalar.activation(out=Ex[:], in_=L[:],
                                     func=mybir.ActivationFunctionType.Exp,
                                     bias=negm[:, 0:1], accum_out=ssum[:])
                lns = finp.tile([128, 1], f32, tag="lns")
                nc.scalar.activation(out=lns[:], in_=ssum[:],
                                     func=mybir.ActivationFunctionType.Ln)
                O = finp.tile([128, 10], f32, tag="O")
                nc.vector.tensor_scalar(out=O[:], in0=L[:],
                                        scalar1=m[:, 0:1], scalar2=lns[:, 0:1],
                                        op0=mybir.AluOpType.subtract,
                                        op1=mybir.AluOpType.subtract)
                nc.sync.dma_start(out=out_d[128 * t:128 * (t + 1), :], in_=O[:])
    nc.finalize()
    return nc


def make_in_maps(meta, cheb_w, cheb_b, fc_w, fc_b):
    wcheb = np.ascontiguousarray(
        cheb_w.transpose(1, 0, 2).reshape(D, K * 50)).astype(bf16)
    in_maps = []
    for c in range(NCORES):
        in_maps.append({
            "xp": meta["xp"][c],
            "idx_lo": meta["idx_lo_w"][c],
            "idx_hi": meta["idx_hi_w"][c],
            "dinv_t": meta["dinv_t"][c],
            "m1_t": meta["m1_t"][c],
            "m2_t": meta["m2_t"][c],
            "sdeg_t": meta["sdeg_t"][c],
            "wcheb": wcheb,
            "cbias": cheb_b.reshape(50, 1).astype(np.float32),
            "fcw": fc_w.astype(bf16),
            "fcb_rep": np.tile(fc_b.reshape(1, 10), (128, 1)).astype(np.float32),
            "identf": np.eye(128, dtype=np.float32),
            "identb": np.eye(128, dtype=np.float32).astype(bf16),
        })
    return in_maps


def kernel(x, edge_index, cheb_w, cheb_b, fc_w, fc_b):
    x = np.ascontiguousarray(np.asarray(x, dtype=np.float32))
    cheb_w = np.asarray(cheb_w, dtype=np.float32)
    cheb_b = np.asarray(cheb_b, dtype=np.float32)
    fc_w = np.asarray(fc_w, dtype=np.float32)
    fc_b = np.asarray(fc_b, dtype=np.float32)

    meta = host_prep(x, edge_index)
    nc = build_nc(meta)
    in_maps = make_in_maps(meta, cheb_w, cheb_b, fc_w, fc_b)

    from concourse.bass_utils import run_bass_kernel_spmd
    res = run_bass_kernel_spmd(nc, in_maps, core_ids=list(range(NCORES)))

    out = np.empty((N, 10), dtype=np.float32)
    for c in range(NCORES):
        out[meta["perm"][c]] = res.results[c]["out"][:NPC]
    return out


# revision 26
# speedup vs baseline: 1.3292x; 1.0766x over previous
"""ChebConv(K=5) + Linear + log_softmax GNN kernel for 8 Trainium2 NeuronCores.

Strategy (graph/data parallel, nodes sharded by destination):
 - Nodes are globally sorted by degree (desc) and dealt round-robin to the 8
   cores (global rank r -> core r%8), so every core's tile t holds nodes of
   near-identical degree.
 - The propagation prop(h) = -D^-1/2 A D^-1/2 h is refactored so the device
   only computes raw gather-sums s[dst] = sum t[src] of the row-scaled table
   t_k = D^-1/2 T_k(L~) x; recurrence: t_{k+1} = -2 D^-1 s - t_{k-1}.
 - Each hop, every core gathers its edges' source rows from a replicated
   bf16 table in HBM via gpsimd dma_gather. Calls rotate over all 4 SWDGE
   queues, which lets the 16 SDMA engines overlap the HBM random-read
   latency of different queues' 256B packets (~128 GB/s vs ~45 single-queue).
 - Slots are interleaved (chunk j, partition s = j-th edge of the tile's
   s-th node), so the per-destination segment sums reduce with an in-place
   VectorE pairwise tree over the chunk axis - node-major, no transposes.
 - int16 gather indices address a 32768-row window; the table stores core
   c's nodes at rows 6400c..6400c+6271 with 128 zero spare rows per core.
   Cores' windows: lo = rows [0, 32768), hi = rows [18432, 51200). Each
   node's edges are split lo/hi, with edges from the overlap region
   assigned to balance the two streams (halves the padding).
 - z = sum_k (sqrt(D) t_k) W_k accumulates per hop (PE transpose + matmul);
   epilogue: relu, 50->10 Linear, row-wise log_softmax.
"""
import numpy as np
import ml_dtypes

bf16 = ml_dtypes.bfloat16

# ---------------- problem constants (hardcoded per contract) ---------------
D = 128
K = 5
NCORES = 8
GMAX_TILES = 2          # tiles per gather call group


def _set_dims(n):
    """Derive layout constants from node count (module globals)."""
    global N, NPC, TILES, S, SPC, TPC, TOT_TOK, WB, HI0, LO_PAD, HI_PAD
    N = n
    NPC = N // NCORES
    TILES = (NPC + 127) // 128
    S = TILES * 128
    SPC = TILES + 1              # row stripes per core (data + 1 zero spare)
    TPC = SPC * 128              # table rows per core
    TOT_TOK = NCORES * TPC
    WB = max(TOT_TOK - 32768, 0)         # hi window base row
    HI0 = min(32768, TOT_TOK)            # lo window end
    assert WB % 128 == 0
    LO_PAD = S                           # row S (core0 spare) = zero row
    HI_PAD = (NCORES - 1) * TPC + S - WB  # core7 spare, hi-window relative
    assert LO_CORES_OK()


def LO_CORES_OK():
    return LO_PAD < HI0 and 0 <= HI_PAD < 32768


_set_dims(50000)


def host_prep(x, edge_index):
    row = np.ascontiguousarray(edge_index[0]).astype(np.int64)
    col = np.ascontiguousarray(edge_index[1]).astype(np.int64)
    deg = np.bincount(row, minlength=N)
    assert (deg > 0).all(), "kernel assumes no isolated (deg-0) nodes"
    degf = deg.astype(np.float32)
    dinv = (1.0 / np.sqrt(degf)).astype(np.float32)

    grank = np.argsort(-deg, kind="stable")
    rank_of = np.empty(N, dtype=np.int64)
    rank_of[grank] = np.arange(N)
    core_of = (rank_of % NCORES).astype(np.int64)
    ric = rank_of // NCORES                  # rank in core, 0..NPC-1
    token = TPC * core_of + ric              # table row per node
    tile_of = ric // 128
    part_of = ric % 128

    # lo/hi split: forced by window coverage; flex edges balance the streams
    st = token[col]
    cat = np.where(st < WB, 0, np.where(st >= HI0, 2, 1))
    forced_lo = np.bincount(row[cat == 0], minlength=N)
    forced_hi = np.bincount(row[cat == 2], minlength=N)
    lo_n = np.clip((deg + 1) // 2, forced_lo, deg - forced_hi)

    order_e = np.argsort(row, kind="stable")
    row_s, col_s, cat_s = row[order_e], col[order_e], cat[order_e]

    def occ(dst_sub, counts):
        stt = np.zeros(N + 1, dtype=np.int64)
        np.cumsum(counts, out=stt[1:])
        return np.arange(dst_sub.shape[0], dtype=np.int64) - stt[dst_sub]

    flex_rank = np.full(len(row_s), -1, dtype=np.int64)
    fm = cat_s == 1
    flex_rank[fm] = occ(row_s[fm], np.bincount(row_s[fm], minlength=N))
    is_lo_e = (cat_s == 0) | (fm & (flex_rank < (lo_n - forced_lo)[row_s]))
    hi_n = deg - lo_n

    # per-tile chunk counts (max over the tile's 128 nodes, all cores)
    C_lo = np.zeros(TILES, dtype=np.int64)
    C_hi = np.zeros(TILES, dtype=np.int64)
    np.maximum.at(C_lo, tile_of, lo_n)
    np.maximum.at(C_hi, tile_of, hi_n)
    C_lo = np.maximum(C_lo, 1)
    C_hi = np.maximum(C_hi, 1)

    # group tiles into gather calls: pair big-C with small-C tiles so all
    # calls are similar size (tiles are degree-sorted, so pair ends)
    groups = [[t, TILES - 1 - t] for t in range(TILES // 2)]
    if TILES % 2:
        groups.append([TILES // 2])

    def call_layout(C):
        tile_chunk0 = np.zeros(TILES, dtype=np.int64)   # chunk offset in call
        call_base = np.zeros(len(groups) + 1, dtype=np.int64)  # slot base
        call_len = np.zeros(len(groups), dtype=np.int64)
        for gi, g in enumerate(groups):
            q = 0
            for t in g:
                tile_chunk0[t] = q
                q += int(C[t])
            call_len[gi] = 128 * q
            call_base[gi + 1] = call_base[gi] + call_len[gi]
        return tile_chunk0, call_base, call_len

    lo_c0, lo_cbase, lo_clen = call_layout(C_lo)
    hi_c0, hi_cbase, hi_clen = call_layout(C_hi)
    n_lo, n_hi = int(lo_cbase[-1]), int(hi_cbase[-1])
    gid_of = np.empty(TILES, dtype=np.int64)
    for gi, g in enumerate(groups):
        for t in g:
            gid_of[t] = gi

    # per-node slot base: call_base + 128*(tile_chunk0) + partition
    node_lo_base = lo_cbase[gid_of[tile_of]] + 128 * lo_c0[tile_of] + part_of
    node_hi_base = hi_cbase[gid_of[tile_of]] + 128 * hi_c0[tile_of] + part_of

    dst_lo, src_lo = row_s[is_lo_e], col_s[is_lo_e]
    dst_hi, src_hi = row_s[~is_lo_e], col_s[~is_lo_e]
    j_lo = occ(dst_lo, np.bincount(dst_lo, minlength=N))
    j_hi = occ(dst_hi, np.bincount(dst_hi, minlength=N))

    idx_lo = np.full((NCORES, n_lo), LO_PAD, dtype=np.int16)
    idx_hi = np.full((NCORES, n_hi), HI_PAD, dtype=np.int16)
    v_lo = token[src_lo]
    v_hi = token[src_hi] - WB
    assert v_lo.max() < 32768 and v_lo.min() >= 0
    assert v_hi.max() < 32768 and v_hi.min() >= 0
    idx_lo[core_of[dst_lo], node_lo_base[dst_lo] + 128 * j_lo] = \
        v_lo.astype(np.int16)
    idx_hi[core_of[dst_hi], node_hi_base[dst_hi] + 128 * j_hi] = \
        v_hi.astype(np.int16)

    # wrap each call's indices to the dma_gather [128, n/16] layout
    def wrap_calls(idx, cbase):
        ncol = idx.shape[1] // 16
        out = np.empty((NCORES, 128, ncol), dtype=np.int16)
        for gi in range(len(groups)):
            a, b = int(cbase[gi]), int(cbase[gi + 1])
            seg = idx[:, a:b].reshape(NCORES, -1, 16).transpose(0, 2, 1)
            out[:, :16, a // 16:b // 16] = seg
        out[:, 16:, :] = np.tile(out[:, :16, :], (1, 7, 1))
        return np.ascontiguousarray(out)

    idx_lo_w = wrap_calls(idx_lo, lo_cbase)
    idx_hi_w = wrap_calls(idx_hi, hi_cbase)

    def rowconst(vals):
        out = np.zeros((NCORES, 128, TILES), dtype=np.float32)
        out[core_of, part_of, tile_of] = vals
        return np.ascontiguousarray(out)

    di2 = dinv * dinv
    xp = np.zeros((NCORES, S, D), dtype=np.float32)
    xp[core_of, ric] = x
    perm = np.empty((NCORES, NPC), dtype=np.int64)
    perm[core_of, ric] = np.arange(N)

    return dict(
        perm=perm, C_lo=C_lo, C_hi=C_hi, groups=groups,
        lo_c0=lo_c0, lo_cbase=lo_cbase, lo_clen=lo_clen,
        hi_c0=hi_c0, hi_cbase=hi_cbase, hi_clen=hi_clen,
        n_lo=n_lo, n_hi=n_hi,
        idx_lo_w=idx_lo_w, idx_hi_w=idx_hi_w,
        xp=xp,
        dinv_t=rowconst(dinv),
        m1_t=rowconst(-di2),
        m2_t=rowconst(-2.0 * di2),
        sdeg_t=rowconst(np.sqrt(degf)),
    )


def build_nc(meta):
    from concourse import bacc, mybir
    import concourse.tile as tile

    f32, bft, i16 = mybir.dt.float32, mybir.dt.bfloat16, mybir.dt.int16
    C_lo, C_hi, groups = meta["C_lo"], meta["C_hi"], meta["groups"]
    n_lo, n_hi = meta["n_lo"], meta["n_hi"]
    CG = int(max(meta["lo_clen"].max(), meta["hi_clen"].max())) // 128

    nc = bacc.Bacc(target_bir_lowering=False, num_swdge_queues=4,
                   dynamic_dma_scratch_size=32768)

    # ---- I/O --------------------------------------------------------------
    xp_d = nc.declare_dram_parameter("xp", [S, D], f32, isOutput=False)
    il_d = nc.declare_dram_parameter("idx_lo", [128, n_lo // 16], i16,
                                     isOutput=False)
    ih_d = nc.declare_dram_parameter("idx_hi", [128, n_hi // 16], i16,
                                     isOutput=False)
    dinv_d = nc.declare_dram_parameter("dinv_t", [128, TILES], f32, isOutput=False)
    m1_d = nc.declare_dram_parameter("m1_t", [128, TILES], f32, isOutput=False)
    m2_d = nc.declare_dram_parameter("m2_t", [128, TILES], f32, isOutput=False)
    sdeg_d = nc.declare_dram_parameter("sdeg_t", [128, TILES], f32, isOutput=False)
    wch_d = nc.declare_dram_parameter("wcheb", [128, K * 50], bft, isOutput=False)
    cb_d = nc.declare_dram_parameter("cbias", [50, 1], f32, isOutput=False)
    fw_d = nc.declare_dram_parameter("fcw", [50, 10], bft, isOutput=False)
    fb_d = nc.declare_dram_parameter("fcb_rep", [128, 10], f32, isOutput=False)
    idf_d = nc.declare_dram_parameter("identf", [128, 128], f32, isOutput=False)
    idb_d = nc.declare_dram_parameter("identb", [128, 128], bft, isOutput=False)
    out_d = nc.declare_dram_parameter("out", [S, 10], f32, isOutput=True)

    # ---- internal DRAM ----------------------------------------------------
    agin = [nc.dram_tensor(f"agin{h}", [TPC, D], bft) for h in range(K - 1)]
    tbl = [nc.dram_tensor(f"tbl{h}", [TOT_TOK, D], bft, addr_space="Shared")
           for h in range(K - 1)]

    with tile.TileContext(nc) as tc:
        with tc.tile_pool(name="cst", bufs=1) as cst, \
             tc.tile_pool(name="xt", bufs=3) as xtp, \
             tc.tile_pool(name="gth", bufs=9) as gthp, \
             tc.tile_pool(name="st", bufs=3) as stp, \
             tc.tile_pool(name="sc", bufs=4) as scp, \
             tc.tile_pool(name="fin", bufs=2) as finp, \
             tc.tile_pool(name="ps_zt", bufs=2, space="PSUM") as ps_zt, \
             tc.tile_pool(name="ps_z", bufs=2, space="PSUM") as ps_z, \
             tc.tile_pool(name="ps_e", bufs=1, space="PSUM") as ps_e:

            # ---- resident constants --------------------------------------
            idx_lo_s = cst.tile([128, n_lo // 16], i16)
            nc.sync.dma_start(out=idx_lo_s[:], in_=il_d[:, :])
            idx_hi_s = cst.tile([128, n_hi // 16], i16)
            nc.sync.dma_start(out=idx_hi_s[:], in_=ih_d[:, :])
            dinv_s = cst.tile([128, TILES], f32)
            nc.sync.dma_start(out=dinv_s[:], in_=dinv_d[:, :])
            m1_s = cst.tile([128, TILES], f32)
            nc.sync.dma_start(out=m1_s[:], in_=m1_d[:, :])
            m2_s = cst.tile([128, TILES], f32)
            nc.sync.dma_start(out=m2_s[:], in_=m2_d[:, :])
            sdeg_s = cst.tile([128, TILES], f32)
            nc.sync.dma_start(out=sdeg_s[:], in_=sdeg_d[:, :])
            wch_s = cst.tile([128, K * 50], bft)
            nc.sync.dma_start(out=wch_s[:], in_=wch_d[:, :])
            cb_s = cst.tile([50, 1], f32)
            nc.sync.dma_start(out=cb_s[:], in_=cb_d[:, :])
            fw_s = cst.tile([50, 10], bft)
            nc.sync.dma_start(out=fw_s[:], in_=fw_d[:, :])
            fb_s = cst.tile([128, 10], f32)
            nc.sync.dma_start(out=fb_s[:], in_=fb_d[:, :])
            idf = cst.tile([128, 128], f32)
            nc.sync.dma_start(out=idf[:], in_=idf_d[:, :])
            idb = cst.tile([128, 128], bft)
            nc.sync.dma_start(out=idb[:], in_=idb_d[:, :])

            prevA = cst.tile([128, SPC, 128], bft)     # node-major t storage
            prevB = cst.tile([128, SPC, 128], bft)
            z_s = cst.tile([50, S], bft)               # z^T accumulator

            nc.vector.memset(prevA[:, TILES, :], 0.0)  # spare rows = zeros
            nc.vector.memset(prevB[:, TILES, :], 0.0)

            def z_project(k, src_bf, t):
                """z^T[:, tile t] (+)= W_k^T @ (sdeg * src)^T  (src node-major)"""
                zsc = scp.tile([128, 128], bft, tag="zsc")
                nc.vector.tensor_scalar_mul(
                    out=zsc[:], in0=src_bf, scalar1=sdeg_s[:, t:t + 1])
                ztp = ps_zt.tile([128, 128], bft, space="PSUM", tag="ztp")
                nc.tensor.transpose(out=ztp[:], in_=zsc[:], identity=idb[:])
                zrhs = scp.tile([128, 128], bft, tag="zrhs")
                nc.scalar.activation(out=zrhs[:], in_=ztp[:],
                                     func=mybir.ActivationFunctionType.Copy)
                zp = ps_z.tile([50, 128], f32, space="PSUM", tag="zp")
                nc.tensor.matmul(out=zp[:], lhsT=wch_s[:, 50 * k:50 * (k + 1)],
                                 rhs=zrhs[:], start=True, stop=True)
                zsl = z_s[:, 128 * t:128 * (t + 1)]
                if k == 0:
                    nc.vector.tensor_copy(out=zsl, in_=zp[:])
                else:
                    nc.vector.tensor_tensor(
                        out=zsl, in0=zsl, in1=zp[:], op=mybir.AluOpType.add)

            def tree(buf, q0, C):
                """in-place pairwise sum of chunks [q0, q0+C) -> chunk q0"""
                n = C
                while n > 1:
                    h = (n + 1) // 2
                    nc.vector.tensor_tensor(
                        out=buf[:, q0:q0 + n - h, :],
                        in0=buf[:, q0:q0 + n - h, :],
                        in1=buf[:, q0 + h:q0 + n, :],
                        op=mybir.AluOpType.add)
                    n = h

            # ---- prologue: t_0 = dinv * x --------------------------------
            for t in range(TILES):
                xt = xtp.tile([128, D], f32)
                nc.sync.dma_start(out=xt[:], in_=xp_d[128 * t:128 * (t + 1), :])
                t0b = stp.tile([128, D], bft, tag="nb")
                nc.vector.tensor_scalar_mul(
                    out=t0b[:], in0=xt[:], scalar1=dinv_s[:, t:t + 1])
                nc.scalar.activation(out=prevA[:, t, :], in_=t0b[:],
                                     func=mybir.ActivationFunctionType.Copy)
                z_project(0, t0b[:], t)
            nc.sync.dma_start(
                out=agin[0][:, :].rearrange("(s p) d -> p s d", p=128),
                in_=prevA[:, :, :])
            nc.gpsimd.collective_compute(
                "AllGather", mybir.AluOpType.bypass,
                replica_groups=[list(range(NCORES))],
                ins=[agin[0][:, :]], outs=[tbl[0][:, :]],
            )

            # ---- hops ----------------------------------------------------
            nq = [0]  # emitted-gather counter (queue = count % 4 keeps the
            # tile framework's 8 DMASW lanes aligned with queues)
            for h in range(1, K):
                tb = tbl[h - 1]
                rd = prevA if h % 2 == 0 else prevB
                wr = prevB if h == 1 else rd

                for gi, g in enumerate(groups):
                    glo = gthp.tile([128, CG, 128], bft, tag="g")
                    ghi = gthp.tile([128, CG, 128], bft, tag="g")
                    nlo = int(meta["lo_clen"][gi])
                    nhi = int(meta["hi_clen"][gi])
                    a = int(meta["lo_cbase"][gi]) // 16
                    nc.gpsimd.dma_gather(
                        out_ap=glo[:, 0:nlo // 128, :], in_ap=tb[0:HI0, :],
                        idxs_ap=idx_lo_s[:, a:a + nlo // 16],
                        num_idxs=nlo, num_idxs_reg=nlo,
                        elem_size=D, queue_num=nq[0] % 4, single_packet=False)
                    nq[0] += 1
                    a = int(meta["hi_cbase"][gi]) // 16
                    nc.gpsimd.dma_gather(
                        out_ap=ghi[:, 0:nhi // 128, :], in_ap=tb[WB:TOT_TOK, :],
                        idxs_ap=idx_hi_s[:, a:a + nhi // 16],
                        num_idxs=nhi, num_idxs_reg=nhi,
                        elem_size=D, queue_num=nq[0] % 4, single_packet=False)
                    nq[0] += 1

                    for t in g:
                        cl, ch = int(C_lo[t]), int(C_hi[t])
                        q0l, q0h = int(meta["lo_c0"][t]), int(meta["hi_c0"][t])
                        tree(glo, q0l, cl)
                        tree(ghi, q0h, ch)
                        stt = stp.tile([128, 128], f32, tag="stt")
                        nc.vector.tensor_tensor(
                            out=stt[:], in0=glo[:, q0l, :], in1=ghi[:, q0h, :],
                            op=mybir.AluOpType.add)
                        nb = stp.tile([128, D], bft, tag="nb")
                        if h == 1:
                            nc.vector.tensor_scalar_mul(
                                out=nb[:], in0=stt[:], scalar1=m1_s[:, t:t + 1])
                        else:
                            r1 = stp.tile([128, 128], f32, tag="r1")
                            nc.vector.tensor_scalar_mul(
                                out=r1[:], in0=stt[:], scalar1=m2_s[:, t:t + 1])
                            nc.vector.tensor_tensor(
                                out=nb[:], in0=r1[:], in1=rd[:, t, :],
                                op=mybir.AluOpType.subtract)
                        if h < K - 1:
                            nc.scalar.activation(
                                out=wr[:, t, :], in_=nb[:],
                                func=mybir.ActivationFunctionType.Copy)
                            # stream this tile's agin slice now so the
                            # AllGather launch isn't gated on a bulk copy
                            nc.sync.dma_start(
                                out=agin[h][128 * t:128 * (t + 1), :]
                                .rearrange("(s p) d -> p s d", p=128),
                                in_=wr[:, t:t + 1, :])
                        z_project(h, nb[:], t)

                if h < K - 1:
                    # spare rows (zeros) complete the agin buffer
                    nc.sync.dma_start(
                        out=agin[h][128 * TILES:TPC, :]
                        .rearrange("(s p) d -> p s d", p=128),
                        in_=prevA[:, TILES:TILES + 1, :])
                    nc.gpsimd.collective_compute(
                        "AllGather", mybir.AluOpType.bypass,
                        replica_groups=[list(range(NCORES))],
                        ins=[agin[h][:, :]], outs=[tbl[h][:, :]],
                    )

            # ---- final: relu, fc, log_softmax ----------------------------
            for t in range(TILES):
                zsl = z_s[:, 128 * t:128 * (t + 1)]
                hT = finp.tile([50, 128], bft, tag="hT")
                nc.scalar.activation(out=hT[:], in_=zsl,
                                     func=mybir.ActivationFunctionType.Relu,
                                     bias=cb_s[:, 0:1])
                lgp = ps_e.tile([10, 128], f32, space="PSUM", tag="lgp")
                nc.tensor.matmul(out=lgp[:], lhsT=fw_s[:], rhs=hT[:],
                                 start=True, stop=True)
                lgs = finp.tile([10, 128], f32, tag="lgs")
                nc.vector.tensor_copy(out=lgs[:], in_=lgp[:])
                ltp = ps_e.tile([128, 10], f32, space="PSUM", tag="ltp")
                nc.tensor.transpose(out=ltp[:], in_=lgs[:],
                                    identity=idf[0:10, 0:10])
                L = finp.tile([128, 10], f32, tag="L")
                nc.vector.tensor_tensor(out=L[:], in0=ltp[:], in1=fb_s[:],
                                        op=mybir.AluOpType.add)
                m = finp.tile([128, 1], f32, tag="m")
                nc.vector.tensor_reduce(out=m[:], in_=L[:],
                                        axis=mybir.AxisListType.X,
                                        op=mybir.AluOpType.max)
                negm = finp.tile([128, 1], f32, tag="negm")
                nc.vector.tensor_scalar_mul(out=negm[:], in0=m[:], scalar1=-1.0)
                Ex = finp.tile([128, 10], f32, tag="Ex")
                ssum = finp.tile([128, 1], f32, tag="ssum")
                nc.scalar.activation(out=Ex[:], in_=L[:],
                                     func=mybir.ActivationFunctionType.Exp,
                                     bias=negm[:, 0:1], accum_out=ssum[:])
                lns = finp.tile([128, 1], f32, tag="lns")
                nc.scalar.activation(out=lns[:], in_=ssum[:],
                                     func=mybir.ActivationFunctionType.Ln)
                O = finp.tile([128, 10], f32, tag="O")
                nc.vector.tensor_scalar(out=O[:], in0=L[:],
                                        scalar1=m[:, 0:1], scalar2=lns[:, 0:1],
                                        op0=mybir.AluOpType.subtract,
                                        op1=mybir.AluOpType.subtract)
                nc.sync.dma_start(out=out_d[128 * t:128 * (t + 1), :], in_=O[:])
    nc.finalize()
    return nc


def make_in_maps(meta, cheb_w, cheb_b, fc_w, fc_b):
    wcheb = np.ascontiguousarray(
        cheb_w.transpose(1, 0, 2).reshape(D, K * 50)).astype(bf16)
    in_maps = []
    for c in range(NCORES):
        in_maps.append({
            "xp": meta["xp"][c],
            "idx_lo": meta["idx_lo_w"][c],
            "idx_hi": meta["idx_hi_w"][c],
            "dinv_t": meta["dinv_t"][c],
            "m1_t": meta["m1_t"][c],
            "m2_t": meta["m2_t"][c],
            "sdeg_t": meta["sdeg_t"][c],
            "wcheb": wcheb,
            "cbias": cheb_b.reshape(50, 1).astype(np.float32),
            "fcw": fc_w.astype(bf16),
            "fcb_rep": np.tile(fc_b.reshape(1, 10), (128, 1)).astype(np.float32),
            "identf": np.eye(128, dtype=np.float32),
            "identb": np.eye(128, dtype=np.float32).astype(bf16),
        })
    return in_maps


def kernel(x, edge_index, cheb_w, cheb_b, fc_w, fc_b):
    x = np.ascontiguousarray(np.asarray(x, dtype=np.float32))
    cheb_w = np.asarray(cheb_w, dtype=np.float32)
    cheb_b = np.asarray(cheb_b, dtype=np.float32)
    fc_w = np.asarray(fc_w, dtype=np.float32)
    fc_b = np.asarray(fc_b, dtype=np.float32)

    meta = host_prep(x, edge_index)
    nc = build_nc(meta)
    in_maps = make_in_maps(meta, cheb_w, cheb_b, fc_w, fc_b)

    from concourse.bass_utils import run_bass_kernel_spmd
    res = run_bass_kernel_spmd(nc, in_maps, core_ids=list(range(NCORES)))

    out = np.empty((N, 10), dtype=np.float32)
    for c in range(NCORES):
        out[meta["perm"][c]] = res.results[c]["out"][:NPC]
    return out


# revision 27
# speedup vs baseline: 1.3546x; 1.0191x over previous
"""ChebConv(K=5) + Linear + log_softmax GNN kernel for 8 Trainium2 NeuronCores.

Strategy (graph/data parallel, nodes sharded by destination):
 - Nodes are globally sorted by degree (desc) and dealt round-robin to the 8
   cores (global rank r -> core r%8), so every core's tile t holds nodes of
   near-identical degree.
 - The propagation prop(h) = -D^-1/2 A D^-1/2 h is refactored so the device
   only computes raw gather-sums s[dst] = sum t[src] of the row-scaled table
   t_k = D^-1/2 T_k(L~) x; recurrence: t_{k+1} = -2 D^-1 s - t_{k-1}.
 - Each hop, every core gathers its edges' source rows from a replicated
   bf16 table in HBM via gpsimd dma_gather. Calls rotate over all 4 SWDGE
   queues, which lets the 16 SDMA engines overlap the HBM random-read
   latency of different queues' 256B packets (~128 GB/s vs ~45 single-queue).
 - Slots are interleaved (chunk j, partition s = j-th edge of the tile's
   s-th node), so the per-destination segment sums reduce with an in-place
   VectorE pairwise tree over the chunk axis - node-major, no transposes.
 - int16 gather indices address a 32768-row window; the table stores core
   c's nodes at rows 6400c..6400c+6271 with 128 zero spare rows per core.
   Cores' windows: lo = rows [0, 32768), hi = rows [18432, 51200). Each
   node's edges are split lo/hi, with edges from the overlap region
   assigned to balance the two streams (halves the padding).
 - z = sum_k (sqrt(D) t_k) W_k accumulates per hop (PE transpose + matmul);
   epilogue: relu, 50->10 Linear, row-wise log_softmax.
"""
import numpy as np
import ml_dtypes

bf16 = ml_dtypes.bfloat16

# ---------------- problem constants (hardcoded per contract) ---------------
D = 128
K = 5
NCORES = 8
GMAX_TILES = 2          # tiles per gather call group


def _set_dims(n):
    """Derive layout constants from node count (module globals)."""
    global N, NPC, TILES, S, SPC, TPC, TOT_TOK, WB, HI0, LO_PAD, HI_PAD
    N = n
    NPC = N // NCORES
    TILES = (NPC + 127) // 128
    S = TILES * 128
    SPC = TILES + 1              # row stripes per core (data + 1 zero spare)
    TPC = SPC * 128              # table rows per core
    TOT_TOK = NCORES * TPC
    WB = max(TOT_TOK - 32768, 0)         # hi window base row
    HI0 = min(32768, TOT_TOK)            # lo window end
    assert WB % 128 == 0
    LO_PAD = S                           # row S (core0 spare) = zero row
    HI_PAD = (NCORES - 1) * TPC + S - WB  # core7 spare, hi-window relative
    assert LO_CORES_OK()


def LO_CORES_OK():
    return LO_PAD < HI0 and 0 <= HI_PAD < 32768


_set_dims(50000)


def host_prep(x, edge_index):
    row = np.ascontiguousarray(edge_index[0]).astype(np.int64)
    col = np.ascontiguousarray(edge_index[1]).astype(np.int64)
    deg = np.bincount(row, minlength=N)
    assert (deg > 0).all(), "kernel assumes no isolated (deg-0) nodes"
    degf = deg.astype(np.float32)
    dinv = (1.0 / np.sqrt(degf)).astype(np.float32)

    grank = np.argsort(-deg, kind="stable")
    rank_of = np.empty(N, dtype=np.int64)
    rank_of[grank] = np.arange(N)
    core_of = (rank_of % NCORES).astype(np.int64)
    ric = rank_of // NCORES                  # rank in core, 0..NPC-1
    token = TPC * core_of + ric              # table row per node
    tile_of = ric // 128
    part_of = ric % 128

    # lo/hi split: forced by window coverage; flex edges balance the streams
    st = token[col]
    cat = np.where(st < WB, 0, np.where(st >= HI0, 2, 1))
    forced_lo = np.bincount(row[cat == 0], minlength=N)
    forced_hi = np.bincount(row[cat == 2], minlength=N)
    lo_n = np.clip((deg + 1) // 2, forced_lo, deg - forced_hi)

    order_e = np.argsort(row, kind="stable")
    row_s, col_s, cat_s = row[order_e], col[order_e], cat[order_e]

    def occ(dst_sub, counts):
        stt = np.zeros(N + 1, dtype=np.int64)
        np.cumsum(counts, out=stt[1:])
        return np.arange(dst_sub.shape[0], dtype=np.int64) - stt[dst_sub]

    flex_rank = np.full(len(row_s), -1, dtype=np.int64)
    fm = cat_s == 1
    flex_rank[fm] = occ(row_s[fm], np.bincount(row_s[fm], minlength=N))
    is_lo_e = (cat_s == 0) | (fm & (flex_rank < (lo_n - forced_lo)[row_s]))
    hi_n = deg - lo_n

    # per-tile chunk counts (max over the tile's 128 nodes, all cores)
    C_lo = np.zeros(TILES, dtype=np.int64)
    C_hi = np.zeros(TILES, dtype=np.int64)
    np.maximum.at(C_lo, tile_of, lo_n)
    np.maximum.at(C_hi, tile_of, hi_n)
    C_lo = np.maximum(C_lo, 1)
    C_hi = np.maximum(C_hi, 1)

    # group tiles into gather calls: pair big-C with small-C tiles so all
    # calls are similar size (tiles are degree-sorted, so pair ends)
    groups = [[t, TILES - 1 - t] for t in range(TILES // 2)]
    if TILES % 2:
        groups.append([TILES // 2])

    def call_layout(C):
        tile_chunk0 = np.zeros(TILES, dtype=np.int64)   # chunk offset in call
        call_base = np.zeros(len(groups) + 1, dtype=np.int64)  # slot base
        call_len = np.zeros(len(groups), dtype=np.int64)
        for gi, g in enumerate(groups):
            q = 0
            for t in g:
                tile_chunk0[t] = q
                q += int(C[t])
            call_len[gi] = 128 * q
            call_base[gi + 1] = call_base[gi] + call_len[gi]
        return tile_chunk0, call_base, call_len

    lo_c0, lo_cbase, lo_clen = call_layout(C_lo)
    hi_c0, hi_cbase, hi_clen = call_layout(C_hi)
    n_lo, n_hi = int(lo_cbase[-1]), int(hi_cbase[-1])
    gid_of = np.empty(TILES, dtype=np.int64)
    for gi, g in enumerate(groups):
        for t in g:
            gid_of[t] = gi

    # per-node slot base: call_base + 128*(tile_chunk0) + partition
    node_lo_base = lo_cbase[gid_of[tile_of]] + 128 * lo_c0[tile_of] + part_of
    node_hi_base = hi_cbase[gid_of[tile_of]] + 128 * hi_c0[tile_of] + part_of

    dst_lo, src_lo = row_s[is_lo_e], col_s[is_lo_e]
    dst_hi, src_hi = row_s[~is_lo_e], col_s[~is_lo_e]
    j_lo = occ(dst_lo, np.bincount(dst_lo, minlength=N))
    j_hi = occ(dst_hi, np.bincount(dst_hi, minlength=N))

    idx_lo = np.full((NCORES, n_lo), LO_PAD, dtype=np.int16)
    idx_hi = np.full((NCORES, n_hi), HI_PAD, dtype=np.int16)
    v_lo = token[src_lo]
    v_hi = token[src_hi] - WB
    assert v_lo.max() < 32768 and v_lo.min() >= 0
    assert v_hi.max() < 32768 and v_hi.min() >= 0
    idx_lo[core_of[dst_lo], node_lo_base[dst_lo] + 128 * j_lo] = \
        v_lo.astype(np.int16)
    idx_hi[core_of[dst_hi], node_hi_base[dst_hi] + 128 * j_hi] = \
        v_hi.astype(np.int16)

    # wrap each call's indices to the dma_gather [128, n/16] layout
    def wrap_calls(idx, cbase):
        ncol = idx.shape[1] // 16
        out = np.empty((NCORES, 128, ncol), dtype=np.int16)
        for gi in range(len(groups)):
            a, b = int(cbase[gi]), int(cbase[gi + 1])
            seg = idx[:, a:b].reshape(NCORES, -1, 16).transpose(0, 2, 1)
            out[:, :16, a // 16:b // 16] = seg
        out[:, 16:, :] = np.tile(out[:, :16, :], (1, 7, 1))
        return np.ascontiguousarray(out)

    idx_lo_w = wrap_calls(idx_lo, lo_cbase)
    idx_hi_w = wrap_calls(idx_hi, hi_cbase)

    def rowconst(vals):
        out = np.zeros((NCORES, 128, TILES), dtype=np.float32)
        out[core_of, part_of, tile_of] = vals
        return np.ascontiguousarray(out)

    di2 = dinv * dinv
    xp = np.zeros((NCORES, S, D), dtype=np.float32)
    xp[core_of, ric] = x
    perm = np.empty((NCORES, NPC), dtype=np.int64)
    perm[core_of, ric] = np.arange(N)

    return dict(
        perm=perm, C_lo=C_lo, C_hi=C_hi, groups=groups,
        lo_c0=lo_c0, lo_cbase=lo_cbase, lo_clen=lo_clen,
        hi_c0=hi_c0, hi_cbase=hi_cbase, hi_clen=hi_clen,
        n_lo=n_lo, n_hi=n_hi,
        idx_lo_w=idx_lo_w, idx_hi_w=idx_hi_w,
        xp=xp,
        dinv_t=rowconst(dinv),
        m1_t=rowconst(-di2),
        m2_t=rowconst(-2.0 * di2),
        sdeg_t=rowconst(np.sqrt(degf)),
    )


def build_nc(meta):
    from concourse import bacc, mybir
    import concourse.tile as tile

    f32, bft, i16 = mybir.dt.float32, mybir.dt.bfloat16, mybir.dt.int16
    C_lo, C_hi, groups = meta["C_lo"], meta["C_hi"], meta["groups"]
    n_lo, n_hi = meta["n_lo"], meta["n_hi"]
    CG = int(max(meta["lo_clen"].max(), meta["hi_clen"].max())) // 128

    nc = bacc.Bacc(target_bir_lowering=False, num_swdge_queues=4,
                   dynamic_dma_scratch_size=32768)

    # ---- I/O --------------------------------------------------------------
    xp_d = nc.declare_dram_parameter("xp", [S, D], f32, isOutput=False)
    il_d = nc.declare_dram_parameter("idx_lo", [128, n_lo // 16], i16,
                                     isOutput=False)
    ih_d = nc.declare_dram_parameter("idx_hi", [128, n_hi // 16], i16,
                                     isOutput=False)
    dinv_d = nc.declare_dram_parameter("dinv_t", [128, TILES], f32, isOutput=False)
    m1_d = nc.declare_dram_parameter("m1_t", [128, TILES], f32, isOutput=False)
    m2_d = nc.declare_dram_parameter("m2_t", [128, TILES], f32, isOutput=False)
    sdeg_d = nc.declare_dram_parameter("sdeg_t", [128, TILES], f32, isOutput=False)
    wch_d = nc.declare_dram_parameter("wcheb", [128, K * 50], bft, isOutput=False)
    cb_d = nc.declare_dram_parameter("cbias", [50, 1], f32, isOutput=False)
    fw_d = nc.declare_dram_parameter("fcw", [50, 10], bft, isOutput=False)
    fb_d = nc.declare_dram_parameter("fcb_rep", [128, 10], f32, isOutput=False)
    idf_d = nc.declare_dram_parameter("identf", [128, 128], f32, isOutput=False)
    idb_d = nc.declare_dram_parameter("identb", [128, 128], bft, isOutput=False)
    out_d = nc.declare_dram_parameter("out", [S, 10], f32, isOutput=True)

    # ---- internal DRAM ----------------------------------------------------
    agin = [nc.dram_tensor(f"agin{h}", [TPC, D], bft) for h in range(K - 1)]
    tbl = [nc.dram_tensor(f"tbl{h}", [TOT_TOK, D], bft, addr_space="Shared")
           for h in range(K - 1)]

    with tile.TileContext(nc) as tc:
        with tc.tile_pool(name="cst", bufs=1) as cst, \
             tc.tile_pool(name="xt", bufs=3) as xtp, \
             tc.tile_pool(name="gth", bufs=9) as gthp, \
             tc.tile_pool(name="st", bufs=3) as stp, \
             tc.tile_pool(name="sc", bufs=4) as scp, \
             tc.tile_pool(name="fin", bufs=2) as finp, \
             tc.tile_pool(name="ps_zt", bufs=2, space="PSUM") as ps_zt, \
             tc.tile_pool(name="ps_z", bufs=2, space="PSUM") as ps_z, \
             tc.tile_pool(name="ps_e", bufs=1, space="PSUM") as ps_e:

            # ---- resident constants --------------------------------------
            idx_lo_s = cst.tile([128, n_lo // 16], i16)
            nc.sync.dma_start(out=idx_lo_s[:], in_=il_d[:, :])
            idx_hi_s = cst.tile([128, n_hi // 16], i16)
            nc.sync.dma_start(out=idx_hi_s[:], in_=ih_d[:, :])
            dinv_s = cst.tile([128, TILES], f32)
            nc.sync.dma_start(out=dinv_s[:], in_=dinv_d[:, :])
            m1_s = cst.tile([128, TILES], f32)
            nc.sync.dma_start(out=m1_s[:], in_=m1_d[:, :])
            m2_s = cst.tile([128, TILES], f32)
            nc.sync.dma_start(out=m2_s[:], in_=m2_d[:, :])
            sdeg_s = cst.tile([128, TILES], f32)
            nc.sync.dma_start(out=sdeg_s[:], in_=sdeg_d[:, :])
            wch_s = cst.tile([128, K * 50], bft)
            nc.sync.dma_start(out=wch_s[:], in_=wch_d[:, :])
            cb_s = cst.tile([50, 1], f32)
            nc.sync.dma_start(out=cb_s[:], in_=cb_d[:, :])
            fw_s = cst.tile([50, 10], bft)
            nc.sync.dma_start(out=fw_s[:], in_=fw_d[:, :])
            fb_s = cst.tile([128, 10], f32)
            nc.sync.dma_start(out=fb_s[:], in_=fb_d[:, :])
            idf = cst.tile([128, 128], f32)
            nc.sync.dma_start(out=idf[:], in_=idf_d[:, :])
            idb = cst.tile([128, 128], bft)
            nc.sync.dma_start(out=idb[:], in_=idb_d[:, :])

            prevA = cst.tile([128, SPC, 128], bft)     # node-major t storage
            prevB = cst.tile([128, SPC, 128], bft)
            z_s = cst.tile([50, S], bft)               # z^T accumulator

            nc.vector.memset(prevA[:, TILES, :], 0.0)  # spare rows = zeros
            nc.vector.memset(prevB[:, TILES, :], 0.0)

            def z_project(k, src_bf, t):
                """z^T[:, tile t] (+)= W_k^T @ (sdeg * src)^T  (src node-major)"""
                zsc = scp.tile([128, 128], bft, tag="zsc")
                nc.vector.tensor_scalar_mul(
                    out=zsc[:], in0=src_bf, scalar1=sdeg_s[:, t:t + 1])
                ztp = ps_zt.tile([128, 128], bft, space="PSUM", tag="ztp")
                nc.tensor.transpose(out=ztp[:], in_=zsc[:], identity=idb[:])
                zrhs = scp.tile([128, 128], bft, tag="zrhs")
                nc.scalar.activation(out=zrhs[:], in_=ztp[:],
                                     func=mybir.ActivationFunctionType.Copy)
                zp = ps_z.tile([50, 128], f32, space="PSUM", tag="zp")
                nc.tensor.matmul(out=zp[:], lhsT=wch_s[:, 50 * k:50 * (k + 1)],
                                 rhs=zrhs[:], start=True, stop=True)
                zsl = z_s[:, 128 * t:128 * (t + 1)]
                if k == 0:
                    nc.vector.tensor_copy(out=zsl, in_=zp[:])
                else:
                    nc.vector.tensor_tensor(
                        out=zsl, in0=zsl, in1=zp[:], op=mybir.AluOpType.add)

            def tree(buf, q0, C):
                """in-place pairwise sum of chunks [q0, q0+C) -> chunk q0"""
                n = C
                while n > 1:
                    h = (n + 1) // 2
                    nc.vector.tensor_tensor(
                        out=buf[:, q0:q0 + n - h, :],
                        in0=buf[:, q0:q0 + n - h, :],
                        in1=buf[:, q0 + h:q0 + n, :],
                        op=mybir.AluOpType.add)
                    n = h

            # ---- prologue: t_0 = dinv * x --------------------------------
            # write agin0 per tile and launch the AllGather first; the k=0
            # z-projections run after, overlapped with AG0 + hop-1 gathers
            for t in range(TILES):
                xt = xtp.tile([128, D], f32)
                nc.sync.dma_start(out=xt[:], in_=xp_d[128 * t:128 * (t + 1), :])
                t0b = stp.tile([128, D], bft, tag="nb")
                nc.vector.tensor_scalar_mul(
                    out=t0b[:], in0=xt[:], scalar1=dinv_s[:, t:t + 1])
                nc.scalar.activation(out=prevA[:, t, :], in_=t0b[:],
                                     func=mybir.ActivationFunctionType.Copy)
                nc.sync.dma_start(
                    out=agin[0][128 * t:128 * (t + 1), :]
                    .rearrange("(s p) d -> p s d", p=128),
                    in_=prevA[:, t:t + 1, :])
            nc.sync.dma_start(
                out=agin[0][128 * TILES:TPC, :]
                .rearrange("(s p) d -> p s d", p=128),
                in_=prevA[:, TILES:TILES + 1, :])
            nc.gpsimd.collective_compute(
                "AllGather", mybir.AluOpType.bypass,
                replica_groups=[list(range(NCORES))],
                ins=[agin[0][:, :]], outs=[tbl[0][:, :]],
            )
            for t in range(TILES):
                z_project(0, prevA[:, t, :], t)

            # ---- hops ----------------------------------------------------
            nq = [0]  # emitted-gather counter (queue = count % 4 keeps the
            # tile framework's 8 DMASW lanes aligned with queues)
            for h in range(1, K):
                tb = tbl[h - 1]
                rd = prevA if h % 2 == 0 else prevB
                wr = prevB if h == 1 else rd

                for gi, g in enumerate(groups):
                    glo = gthp.tile([128, CG, 128], bft, tag="g")
                    ghi = gthp.tile([128, CG, 128], bft, tag="g")
                    nlo = int(meta["lo_clen"][gi])
                    nhi = int(meta["hi_clen"][gi])
                    a = int(meta["lo_cbase"][gi]) // 16
                    nc.gpsimd.dma_gather(
                        out_ap=glo[:, 0:nlo // 128, :], in_ap=tb[0:HI0, :],
                        idxs_ap=idx_lo_s[:, a:a + nlo // 16],
                        num_idxs=nlo, num_idxs_reg=nlo,
                        elem_size=D, queue_num=nq[0] % 4, single_packet=False)
                    nq[0] += 1
                    a = int(meta["hi_cbase"][gi]) // 16
                    nc.gpsimd.dma_gather(
                        out_ap=ghi[:, 0:nhi // 128, :], in_ap=tb[WB:TOT_TOK, :],
                        idxs_ap=idx_hi_s[:, a:a + nhi // 16],
                        num_idxs=nhi, num_idxs_reg=nhi,
                        elem_size=D, queue_num=nq[0] % 4, single_packet=False)
                    nq[0] += 1

                    for t in g:
                        cl, ch = int(C_lo[t]), int(C_hi[t])
                        q0l, q0h = int(meta["lo_c0"][t]), int(meta["hi_c0"][t])
                        tree(glo, q0l, cl)
                        tree(ghi, q0h, ch)
                        stt = stp.tile([128, 128], f32, tag="stt")
                        nc.vector.tensor_tensor(
                            out=stt[:], in0=glo[:, q0l, :], in1=ghi[:, q0h, :],
                            op=mybir.AluOpType.add)
                        nb = stp.tile([128, D], bft, tag="nb")
                        if h == 1:
                            nc.vector.tensor_scalar_mul(
                                out=nb[:], in0=stt[:], scalar1=m1_s[:, t:t + 1])
                        else:
                            r1 = stp.tile([128, 128], f32, tag="r1")
                            nc.vector.tensor_scalar_mul(
                                out=r1[:], in0=stt[:], scalar1=m2_s[:, t:t + 1])
                            nc.vector.tensor_tensor(
                                out=nb[:], in0=r1[:], in1=rd[:, t, :],
                                op=mybir.AluOpType.subtract)
                        if h < K - 1:
                            nc.scalar.activation(
                                out=wr[:, t, :], in_=nb[:],
                                func=mybir.ActivationFunctionType.Copy)
                            # stream this tile's agin slice now so the
                            # AllGather launch isn't gated on a bulk copy
                            nc.sync.dma_start(
                                out=agin[h][128 * t:128 * (t + 1), :]
                                .rearrange("(s p) d -> p s d", p=128),
                                in_=wr[:, t:t + 1, :])
                        z_project(h, nb[:], t)

                if h < K - 1:
                    # spare rows (zeros) complete the agin buffer
                    nc.sync.dma_start(
                        out=agin[h][128 * TILES:TPC, :]
                        .rearrange("(s p) d -> p s d", p=128),
                        in_=prevA[:, TILES:TILES + 1, :])
                    nc.gpsimd.collective_compute(
                        "AllGather", mybir.AluOpType.bypass,
                        replica_groups=[list(range(NCORES))],
                        ins=[agin[h][:, :]], outs=[tbl[h][:, :]],
                    )

            # ---- final: relu, fc, log_softmax ----------------------------
            for t in range(TILES):
                zsl = z_s[:, 128 * t:128 * (t + 1)]
                hT = finp.tile([50, 128], bft, tag="hT")
                nc.scalar.activation(out=hT[:], in_=zsl,
                                     func=mybir.ActivationFunctionType.Relu,
                                     bias=cb_s[:, 0:1])
                lgp = ps_e.tile([10, 128], f32, space="PSUM", tag="lgp")
                nc.tensor.matmul(out=lgp[:], lhsT=fw_s[:], rhs=hT[:],
                                 start=True, stop=True)
                lgs = finp.tile([10, 128], f32, tag="lgs")
                nc.vector.tensor_copy(out=lgs[:], in_=lgp[:])
                ltp = ps_e.tile([128, 10], f32, space="PSUM", tag="ltp")
                nc.tensor.transpose(out=ltp[:], in_=lgs[:],
                                    identity=idf[0:10, 0:10])
                L = finp.tile([128, 10], f32, tag="L")
                nc.vector.tensor_tensor(out=L[:], in0=ltp[:], in1=fb_s[:],
                                        op=mybir.AluOpType.add)
                m = finp.tile([128, 1], f32, tag="m")
                nc.vector.tensor_reduce(out=m[:], in_=L[:],
                                        axis=mybir.AxisListType.X,
                                        op=mybir.AluOpType.max)
                negm = finp.tile([128, 1], f32, tag="negm")
                nc.vector.tensor_scalar_mul(out=negm[:], in0=m[:], scalar1=-1.0)
                Ex = finp.tile([128, 10], f32, tag="Ex")
                ssum = finp.tile([128, 1], f32, tag="ssum")
                nc.scalar.activation(out=Ex[:], in_=L[:],
                                     func=mybir.ActivationFunctionType.Exp,
                                     bias=negm[:, 0:1], accum_out=ssum[:])
                lns = finp.tile([128, 1], f32, tag="lns")
                nc.scalar.activation(out=lns[:], in_=ssum[:],
                                     func=mybir.ActivationFunctionType.Ln)
                O = finp.tile([128, 10], f32, tag="O")
                nc.vector.tensor_scalar(out=O[:], in0=L[:],
                                        scalar1=m[:, 0:1], scalar2=lns[:, 0:1],
                                        op0=mybir.AluOpType.subtract,
                                        op1=mybir.AluOpType.subtract)
                nc.sync.dma_start(out=out_d[128 * t:128 * (t + 1), :], in_=O[:])
    nc.finalize()
    return nc


def make_in_maps(meta, cheb_w, cheb_b, fc_w, fc_b):
    wcheb = np.ascontiguousarray(
        cheb_w.transpose(1, 0, 2).reshape(D, K * 50)).astype(bf16)
    in_maps = []
    for c in range(NCORES):
        in_maps.append({
            "xp": meta["xp"][c],
            "idx_lo": meta["idx_lo_w"][c],
            "idx_hi": meta["idx_hi_w"][c],
            "dinv_t": meta["dinv_t"][c],
            "m1_t": meta["m1_t"][c],
            "m2_t": meta["m2_t"][c],
            "sdeg_t": meta["sdeg_t"][c],
            "wcheb": wcheb,
            "cbias": cheb_b.reshape(50, 1).astype(np.float32),
            "fcw": fc_w.astype(bf16),
            "fcb_rep": np.tile(fc_b.reshape(1, 10), (128, 1)).astype(np.float32),
            "identf": np.eye(128, dtype=np.float32),
            "identb": np.eye(128, dtype=np.float32).astype(bf16),
        })
    return in_maps


def kernel(x, edge_index, cheb_w, cheb_b, fc_w, fc_b):
    x = np.ascontiguousarray(np.asarray(x, dtype=np.float32))
    cheb_w = np.asarray(cheb_w, dtype=np.float32)
    cheb_b = np.asarray(cheb_b, dtype=np.float32)
    fc_w = np.asarray(fc_w, dtype=np.float32)
    fc_b = np.asarray(fc_b, dtype=np.float32)

    meta = host_prep(x, edge_index)
    nc = build_nc(meta)
    in_maps = make_in_maps(meta, cheb_w, cheb_b, fc_w, fc_b)

    from concourse.bass_utils import run_bass_kernel_spmd
    res = run_bass_kernel_spmd(nc, in_maps, core_ids=list(range(NCORES)))

    out = np.empty((N, 10), dtype=np.float32)
    for c in range(NCORES):
        out[meta["perm"][c]] = res.results[c]["out"][:NPC]
    return out
